# revision 22
# baseline (speedup 1.0000x reference)
"""GraphUNet (N=4096, E=65536, C=256, depth 3, ratio 0.5) on 8 trn2 NeuronCores.

Five compiled modules, six launches. Device does all adjacency matmuls
(A@x SpMMs and the dense pooled A@A products); host does O(n^2) prep,
top-k, permutation gathers, CxC weight folds, and scaling-vector algebra
(all folded out of the device programs).

  A  (K1+K4c) row-sharded N0-apply: psum = A0[rows] @ z, z host-split into
     two scaled fp8 halves (DoubleRow), raw f32 psums shipped; host applies
     dis scalings + 2*dis^2 diag term.
  B0 (K2) level-1: (4 row x 2 col)-grid M1 = L@R in fp8 DR; M^T col-blocks
     shipped fp8 (ints, exact); partial GCN P = X'^T @ w (fp8 DR) shipped
     f32; host reduces the 2 k-partials, applies dis/diag corrections+relu.
  B1 (K3) level-2: same at half size; M2 entries >16 so X' ships bf16 and
     the P-chain rhs is fp16.
  C  (K4a) level-3 factored GCN (no M3 materialization): u = R3 @ w3,
     x_rows = L3[rows] @ u; bf16/fp16.
  D  (K4b) both up-GCNs fused: xU1 = relu-scale(M2p^T-chain) written
     straight into the combined rhs tile; v2 = Kst^T @ [w1; xU1s] with the
     unpool-scatter folded into host-gathered Kst = [M1p; M1p[:,p2]]^T fp8;
     transpose + Wu1 matmul + relu on device.

All device inputs are host-packed [128, X] contiguous SBUF images (full
DMA bandwidth, no sub-512B descriptor penalty). All fp8/fp16 payloads are
pre-scaled by power-of-2 to dodge fp8's 2^-10 subnormal floor; scales are
folded into host-side post-processing (everything downstream is linear,
and relu commutes with positive scales).
"""

import numpy as np
import ml_dtypes

from contextlib import ExitStack

import concourse.bass as bass
import concourse.mybir as mybir
import concourse.tile as tile
from concourse import bacc
from concourse.bass_utils import run_bass_kernel_spmd

NCORES = 8
C = 256
F32 = mybir.dt.float32
F16 = mybir.dt.float16
BF16 = mybir.dt.bfloat16
FP8 = mybir.dt.float8e4

NP8 = ml_dtypes.float8_e4m3fn
NP16 = np.float16
NPBF = ml_dtypes.bfloat16

_TRACE = {"on": False, "results": [], "ncs": []}


# ------------------------------------------------------------- host helpers
def _pack(arr, np_dt):
    """[K, F] -> [128, (K//128)*F] image; k-tile o holds rows o*128..o*128+127."""
    K, F = arr.shape
    KT = K // 128
    return np.ascontiguousarray(
        arr.reshape(KT, 128, F).transpose(1, 0, 2).reshape(128, KT * F)
    ).astype(np_dt)


def _unpack(img, MO, F):
    """[128, MO*F] -> [MO*128, F] (inverse of _pack on the output side)."""
    return np.ascontiguousarray(
        img.reshape(128, MO, F).transpose(1, 0, 2).reshape(MO * 128, F))


def _pow2_for(m, target):
    m = float(m)
    return 1.0 if m <= 0 else float(2.0 ** np.floor(np.log2(target / m)))


def _split8(x, alpha):
    v = (x * alpha).astype(np.float32)
    h = v.astype(NP8)
    lo = (v - h.astype(np.float32)).astype(NP8)
    return h, lo


def _mk_dis(deg):
    return (1.0 / np.sqrt(np.maximum(deg, 1e-12))).astype(np.float32)


# ----------------------------------------------------------- device helpers
def _in_img(nc, name, KT, F, dt):
    return nc.dram_tensor(name, [128, KT * F], dt, kind="ExternalInput")


def _ld_chunks(nc, pool, dram, KT, F, tag, chunks):
    """Allocate [128, KT, F] tile; return (tile, chunk-issue fns).
    chunks: list of kt-counts per dma, or an int chunk size."""
    t = pool.tile([128, KT, F], dram.dtype, tag=tag, name=tag)
    r = dram.ap().rearrange("p (o f) -> p o f", f=F)
    if isinstance(chunks, int):
        chunks = [chunks] * ((KT + chunks - 1) // chunks)
    fns, k = [], 0
    for ck in chunks:
        k0, k1 = k, min(KT, k + ck)
        k = k1
        fns.append(lambda k0=k0, k1=k1: nc.sync.dma_start(
            t[:, k0:k1, :], r[:, k0:k1, :]))
        if k >= KT:
            break
    return t, fns


def _interleave(*fn_lists):
    n = max(len(f) for f in fn_lists)
    for i in range(n):
        for fns in fn_lists:
            if i < len(fns):
                fns[i]()


def _spread_copy(nc, idx, dst, src):
    if idx % 2 == 0:
        nc.scalar.copy(dst, src)
    else:
        nc.vector.tensor_copy(dst, src)


def _mm_ktouter(nc, ps, lhsT, rhs_list, M, NF, dr, tagp, consumer,
                stagger=False):
    """kt-outer accumulation: psums for all M//128 row-blocks live at once.
    lhsT [128, KT, M]; each rhs [128, KT, NF]. dr: fp8 DoubleRow.
    stagger: issue the last k-step mo-major with the consumer interleaved, so
    psum copies start as soon as each row-block's accumulation closes."""
    n_mo = M // 128
    KT = lhsT.shape[1]
    pss = [ps.tile([128, NF], F32, tag=f"{tagp}{m}", name=f"{tagp}{m}")
           for m in range(n_mo)]

    def mm(mo, k, ci, start, stop):
        if dr:
            nc.tensor.matmul(
                pss[mo][:],
                lhsT[:, 2 * k:2 * k + 2, mo * 128:(mo + 1) * 128],
                rhs_list[ci][:, 2 * k:2 * k + 2, :],
                start=start, stop=stop,
                perf_mode=mybir.MatmulPerfMode.DoubleRow)
        else:
            nc.tensor.matmul(
                pss[mo][:], lhsT[:, k, mo * 128:(mo + 1) * 128],
                rhs_list[ci][:, k, :], start=start, stop=stop)

    nch = len(rhs_list)
    KS = (KT // 2) if dr else KT
    nk_main = KS - 1 if (stagger and KS > 1) else KS
    step = 0
    for k in range(nk_main):
        for ci in range(nch):
            step += 1
            for mo in range(n_mo):
                mm(mo, k, ci, step == 1, step == KS * nch)
    if nk_main < KS:
        for mo in range(n_mo):
            for ci in range(nch):
                mm(mo, KS - 1, ci, False, ci == nch - 1)
            consumer(mo, pss[mo])
    else:
        for mo in range(n_mo):
            consumer(mo, pss[mo])


def _new_nc():
    return bacc.Bacc("TRN2", target_bir_lowering=False, debug=False,
                     num_devices=NCORES)


def _run(nc, in_maps):
    res = run_bass_kernel_spmd(nc, in_maps, list(range(NCORES)),
                               trace=_TRACE["on"])
    _TRACE["ncs"].append(nc)
    if _TRACE["on"]:
        _TRACE["results"].append(res)
    return res.results


# --------------------------------------------------------------- module A
def build_A():
    """psum[rows_c] = A0[rows_c] @ (zh + zl); rows_c = 512-row slab."""
    nc = _new_nc()
    KT, RW = 32, 512
    AT = _in_img(nc, "AT", KT, RW, FP8)
    zh = _in_img(nc, "zh", KT, C, FP8)
    zl = _in_img(nc, "zl", KT, C, FP8)
    po = nc.dram_tensor("po", [128, (RW // 128) * C], F32, kind="ExternalOutput")
    with tile.TileContext(nc) as tc:
        ctx = ExitStack()
        sb = ctx.enter_context(tc.tile_pool(name="sb", bufs=1))
        ps = ctx.enter_context(tc.tile_pool(name="ps", bufs=1, space="PSUM"))
        AT_sb, a_fns = _ld_chunks(nc, sb, AT, KT, RW, "AT", [6, 6, 6, 6, 4, 2, 2])
        zh_sb, h_fns = _ld_chunks(nc, sb, zh, KT, C, "zh", 8)
        zl_sb, l_fns = _ld_chunks(nc, sb, zl, KT, C, "zl", 8)
        _interleave(a_fns, h_fns, l_fns)
        o_sb = sb.tile([128, RW // 128, C], F32, tag="o", name="o")
        ro = po.ap().rearrange("p (o f) -> p o f", f=C)

        def fin(mo, p):
            _spread_copy(nc, mo, o_sb[:, mo, :], p[:])
            if mo % 2 == 1:
                nc.sync.dma_start(ro[:, mo - 1:mo + 1, :],
                                  o_sb[:, mo - 1:mo + 1, :])

        _mm_ktouter(nc, ps, AT_sb, [zh_sb, zl_sb], RW, C, True, "mp", fin,
                    stagger=True)
        ctx.close()
    nc.compile()
    return nc


# --------------------------------------------------------------- module B
def build_B(NPREV, NK, xdt, wsplit):
    """(4 rows x 2 cols) grid core: X' = M^T[cols_j, rows_i] (fp8 DR chain),
    P = X'^T @ w[cols_j] partial GCN. wsplit=2 -> two fp8 rhs (DR);
    wsplit=1 -> one fp16 rhs."""
    nc = _new_nc()
    KT = NPREV // 128
    CW, RW = NK // 2, NK // 4
    MOX, MOP = CW // 128, RW // 128
    Rc = _in_img(nc, "Rc", KT, CW, FP8)
    LrT = _in_img(nc, "LrT", KT, RW, FP8)
    wdt = FP8 if wsplit == 2 else F16
    ws = [_in_img(nc, f"w{i}", MOX, C, wdt) for i in range(wsplit)]
    XT = nc.dram_tensor("XT", [128, MOX * RW], xdt, kind="ExternalOutput")
    Po = nc.dram_tensor("Po", [128, MOP * C], BF16, kind="ExternalOutput")
    rck = [6, 6, 6, 6, 4, 2, 2] if KT == 32 else [4, 4, 4, 2, 2]
    with tile.TileContext(nc) as tc:
        ctx = ExitStack()
        sb = ctx.enter_context(tc.tile_pool(name="sb", bufs=1))
        ps = ctx.enter_context(tc.tile_pool(name="ps", bufs=1, space="PSUM"))
        Rc_sb, r_fns = _ld_chunks(nc, sb, Rc, KT, CW, "Rc", rck)
        LrT_sb, l_fns = _ld_chunks(nc, sb, LrT, KT, RW, "LrT", rck)
        w_sbs, w_fns = [], []
        for i, w in enumerate(ws):
            t, fns = _ld_chunks(nc, sb, w, MOX, C, f"w{i}", MOX)
            w_sbs.append(t)
            w_fns.append(fns)
        _interleave(r_fns, l_fns, *w_fns)
        X_sb = sb.tile([128, MOX, RW], xdt, tag="X", name="X")
        rx = XT.ap().rearrange("p (o f) -> p o f", f=RW)

        def xfin(mo, p):
            _spread_copy(nc, mo, X_sb[:, mo, :], p[:])
            if mo == MOX // 2 - 1 or mo == MOX - 1:
                nc.sync.dma_start(rx[:, mo - MOX // 2 + 1:mo + 1, :],
                                  X_sb[:, mo - MOX // 2 + 1:mo + 1, :])

        _mm_ktouter(nc, ps, Rc_sb, [LrT_sb], CW, RW, True, "mp", xfin,
                    stagger=True)
        P_sb = sb.tile([128, MOP, C], BF16, tag="P", name="P")

        def pfin(mo, p):
            _spread_copy(nc, mo + 1, P_sb[:, mo, :], p[:])

        _mm_ktouter(nc, ps, X_sb, w_sbs, RW, C, wsplit == 2, "mp", pfin,
                    stagger=True)
        nc.sync.dma_start(Po.ap(), P_sb[:].rearrange("p o f -> p (o f)"))
        ctx.close()
    nc.compile()
    return nc


# --------------------------------------------------------------- module C
def build_C():
    """M3c = L3[rows_c] @ R3 (rides the R3 stream), then transpose and
    x[rows_c] = M3c @ w3; 64 rows/core."""
    nc = _new_nc()
    R3 = _in_img(nc, "R3", 8, 512, BF16)
    w3 = _in_img(nc, "w3", 4, C, F16)
    L3cT = _in_img(nc, "L3cT", 8, 64, BF16)
    ident = nc.dram_tensor("ident", [128, 128], BF16, kind="ExternalInput")
    xo = nc.dram_tensor("xo", [64, C], F32, kind="ExternalOutput")
    with tile.TileContext(nc) as tc:
        ctx = ExitStack()
        sb = ctx.enter_context(tc.tile_pool(name="sb", bufs=1))
        ps = ctx.enter_context(tc.tile_pool(name="ps", bufs=1, space="PSUM"))
        L3_sb, l_fns = _ld_chunks(nc, sb, L3cT, 8, 64, "L3cT", 8)
        id_sb = sb.tile([128, 128], BF16, tag="id", name="id")
        R3_sb, r_fns = _ld_chunks(nc, sb, R3, 8, 512, "R3", [2, 2, 2, 1, 1])
        w3_sb, w_fns = _ld_chunks(nc, sb, w3, 4, C, "w3", 4)
        l_fns[0]()
        r_fns[0]()
        nc.sync.dma_start(id_sb[:], ident.ap())
        _interleave(r_fns[1:], w_fns)
        # M3c = L3c @ R3  [64, 512], kt-outer over the R3 stream
        pm = ps.tile([128, 512], F32, tag="pm", name="pm")
        for kt in range(8):
            nc.tensor.matmul(pm[:64, :], L3_sb[:, kt, :], R3_sb[:, kt, :],
                             start=(kt == 0), stop=(kt == 7))
        m3 = sb.tile([128, 512], BF16, tag="m3", name="m3")
        nc.scalar.copy(m3[:64, :], pm[:64, :])
        m3T = sb.tile([128, 4, 64], BF16, tag="m3T", name="m3T")
        for cc in range(4):
            pt = ps.tile([128, 64], BF16, tag=f"pt{cc % 2}", name="pt")
            nc.tensor.transpose(pt[:, :], m3[:64, cc * 128:(cc + 1) * 128],
                                id_sb[:64, :64])
            _spread_copy(nc, cc, m3T[:, cc, :], pt[:, :])
        px = ps.tile([128, C], F32, tag="px", name="px")
        for kt in range(4):
            nc.tensor.matmul(px[:64, :], m3T[:, kt, :], w3_sb[:, kt, :],
                             start=(kt == 0), stop=(kt == 3))
        o_sb = sb.tile([128, C], F32, tag="o", name="o")
        nc.scalar.copy(o_sb[:64, :], px[:64, :])
        nc.sync.dma_start(xo.ap(), o_sb[:64, :])
        ctx.close()
    nc.compile()
    return nc


# --------------------------------------------------------------- module D
def build_D():
    """xU1 = relu-scale(M2p-chain) -> rhs tile; v2 = Kst^T @ [w1; xU1s];
    xU2[rows_c] = relu((dis1-scaled v2) @ Wu1). 256 v2-rows per core."""
    nc = _new_nc()
    M2pT = _in_img(nc, "M2pT", 8, 1024, BF16)
    wu = _in_img(nc, "wu", 8, C, F16)
    Kst = _in_img(nc, "Kst", 24, C, FP8)
    w1 = _in_img(nc, "w1", 16, C, F16)
    Wu1 = _in_img(nc, "Wu1", 2, C, F16)
    ident = nc.dram_tensor("ident", [128, 128], BF16, kind="ExternalInput")
    svec = nc.dram_tensor("svec", [128, 8], F32, kind="ExternalInput")
    d1vec = nc.dram_tensor("d1vec", [128, 2], F32, kind="ExternalInput")
    xo = nc.dram_tensor("xo", [128, 2 * C], F32, kind="ExternalOutput")
    with tile.TileContext(nc) as tc:
        ctx = ExitStack()
        sb = ctx.enter_context(tc.tile_pool(name="sb", bufs=1))
        ps = ctx.enter_context(tc.tile_pool(name="ps", bufs=1, space="PSUM"))
        M2pT_sb, m_fns = _ld_chunks(nc, sb, M2pT, 8, 1024, "M2pT",
                                    [1, 2, 2, 1, 1, 1])
        wu_sb, wu_fns = _ld_chunks(nc, sb, wu, 8, C, "wu", [2, 3, 3])
        Kst_sb, k_fns = _ld_chunks(nc, sb, Kst, 24, C, "Kst", 6)
        Wu1_sb, wf_fns = _ld_chunks(nc, sb, Wu1, 2, C, "Wu1", 2)
        id_sb = sb.tile([128, 128], BF16, tag="id", name="id")
        sv_sb = sb.tile([128, 8], F32, tag="sv", name="sv")
        d1_sb = sb.tile([128, 2], F32, tag="d1", name="d1")
        rhs_sb = sb.tile([128, 24, C], F16, tag="rhs", name="rhs")

        def aux():
            nc.sync.dma_start(id_sb[:], ident.ap())
            nc.sync.dma_start(sv_sb[:], svec.ap())
            nc.sync.dma_start(d1_sb[:], d1vec.ap())
        r1 = w1.ap().rearrange("p (o f) -> p o f", f=C)
        w1_fns = [lambda k0=k0: nc.sync.dma_start(
            rhs_sb[:, k0:k0 + 4, :], r1[:, k0:k0 + 4, :])
            for k0 in range(0, 16, 4)]
        # Kst/w1 first: the v2 w1-part runs before xU1 needs all 8 psum
        # banks; M2pT/wu stream second with xU1 riding it.
        aux()
        _interleave(k_fns, w1_fns, wf_fns)
        _interleave(m_fns, wu_fns)

        # v2 part 1: Kst k-tiles 0..15 against w1 (rides the first stream);
        # parked in SBUF (pre-scaled by dis1/ac) to free the psum banks.
        kst3 = Kst_sb  # [128, 24, C]
        v2a = sb.tile([128, 2, C], F32, tag="v2a", name="v2a")
        vps = [ps.tile([128, C], F32, tag=f"mp{m}", name=f"v2p{m}")
               for m in range(2)]
        for kt in range(16):
            for mo in range(2):
                nc.tensor.matmul(
                    vps[mo][:], kst3[:, kt, mo * 128:(mo + 1) * 128],
                    rhs_sb[:, kt, :], start=(kt == 0), stop=(kt == 15))
        for mo in range(2):
            if mo % 2 == 0:
                nc.scalar.activation(v2a[:, mo, :], vps[mo][:],
                                     mybir.ActivationFunctionType.Copy,
                                     scale=d1_sb[:, mo:mo + 1])
            else:
                nc.vector.tensor_scalar_mul(v2a[:, mo, :], vps[mo][:],
                                            d1_sb[:, mo:mo + 1])

        # xU1s written straight into rhs tile k-tiles 16..23; spread the
        # relu+scale over Act and DVE so the handoff to v2 isn't serial
        def xufin(mo, p):
            if mo % 2 == 0:
                nc.scalar.activation(rhs_sb[:, 16 + mo, :], p[:],
                                     mybir.ActivationFunctionType.Relu,
                                     scale=sv_sb[:, mo:mo + 1])
            else:
                nc.vector.tensor_scalar(rhs_sb[:, 16 + mo, :], p[:],
                                        sv_sb[:, mo:mo + 1], 0.0,
                                        mybir.AluOpType.mult,
                                        mybir.AluOpType.max)

        _mm_ktouter(nc, ps, M2pT_sb, [wu_sb], 1024, C, False, "mp", xufin,
                    stagger=True)

        # v2 part 2: Kst k-tiles 16..23 against xU1s, then add the parked
        # part-1 result: v2b = psum*d1 + v2a
        v2b = sb.tile([128, 2, C], BF16, tag="v2b", name="v2b")
        v2ps = [ps.tile([128, C], F32, tag=f"mp{m}", name=f"v2q{m}")
                for m in range(2)]
        for kt in range(8):
            for mo in range(2):
                nc.tensor.matmul(
                    v2ps[mo][:], kst3[:, 16 + kt, mo * 128:(mo + 1) * 128],
                    rhs_sb[:, 16 + kt, :], start=(kt == 0), stop=(kt == 7))
        for mo in range(2):
            nc.vector.scalar_tensor_tensor(
                v2b[:, mo, :], v2ps[mo][:], d1_sb[:, mo:mo + 1],
                v2a[:, mo, :], mybir.AluOpType.mult, mybir.AluOpType.add)
        v2T = sb.tile([128, 2, C], BF16, tag="v2T", name="v2T")
        for mo in range(2):
            for cc in range(2):
                pst = ps.tile([128, 128], BF16, tag="mp4", name="pt")
                nc.tensor.transpose(pst[:], v2b[:, mo, cc * 128:(cc + 1) * 128],
                                    id_sb[:])
                _spread_copy(nc, mo, v2T[:, cc, mo * 128:(mo + 1) * 128],
                             pst[:])
        o_sb = sb.tile([128, 2, C], F32, tag="o", name="o")
        ro = xo.ap().rearrange("p (o f) -> p o f", f=C)

        def ofin(mo, p):
            if mo % 2 == 0:
                nc.scalar.activation(o_sb[:, mo, :], p[:],
                                     mybir.ActivationFunctionType.Relu)
            else:
                nc.vector.tensor_scalar_max(o_sb[:, mo, :], p[:], 0.0)
            nc.sync.dma_start(ro[:, mo, :], o_sb[:, mo, :])

        _mm_ktouter(nc, ps, v2T, [Wu1_sb], 256, C, False, "mp", ofin,
                    stagger=True)
        ctx.close()
    nc.compile()
    return nc


# =================================================================== host
def kernel(x, edge_index, W_init, b_init, W_down, b_down, p_pool,
           W_up, b_up, W_final, b_final):
    x = np.asarray(x, np.float32)
    N = x.shape[0]

    A0 = np.zeros((N, N), np.float32)
    np.add.at(A0, (np.asarray(edge_index[0]), np.asarray(edge_index[1])), 1.0)
    assert A0.max() <= 15
    dis0 = _mk_dis(A0.sum(1) + 2.0)
    y0 = x @ np.asarray(W_init, np.float32)
    z = dis0[:, None] * y0

    ncA = build_A()
    ncB0 = build_B(4096, 2048, FP8, 2)
    ncB1 = build_B(2048, 1024, BF16, 1)
    ncC = build_C()
    ncD = build_D()

    # per-core A0 row-slab lhsT images (shared by K1 and K4c)
    AT_imgs = [_pack(np.ascontiguousarray(A0[c * 512:(c + 1) * 512].T), NP8)
               for c in range(NCORES)]

    # ---- K1
    az = _pow2_for(np.abs(z).max(), 128.0)
    zh, zl = _split8(z, az)
    zh_img, zl_img = _pack(zh, NP8), _pack(zl, NP8)
    maps = [{"AT": AT_imgs[c], "zh": zh_img, "zl": zl_img}
            for c in range(NCORES)]
    res = _run(ncA, maps)
    x0 = np.concatenate([_unpack(res[c]["po"], 4, C) for c in range(NCORES)], 0)
    x0 = dis0[:, None] * (x0 / az) + 2.0 * dis0[:, None] ** 2 * y0

    # ---- down levels 0,1 (K2, K3)
    Bh = A0 + np.eye(N, dtype=np.float32)
    xcur = x0
    n = N
    xs = [x0]
    Ms, perms, diss = [], [], [dis0]
    for lev in range(2):
        p = np.asarray(p_pool[lev], np.float32)
        score = (xcur @ p) / np.linalg.norm(p)
        k = n // 2
        perm = np.argsort(-score, kind="stable")[:k]
        sv = score[perm]
        perms.append(perm)
        L = Bh[perm, :]
        R = Bh[:, perm]
        assert Bh.max() <= 15
        diagM = np.einsum('ak,ka->a', L, R, optimize=True).astype(np.float32)
        r = R.sum(1, dtype=np.float64)
        deg = (L @ r.astype(np.float32)).astype(np.float64) - diagM + 2.0
        dis = _mk_dis(deg.astype(np.float32))
        diss.append(dis)
        xp = (xcur[perm] * np.tanh(sv)[:, None]).astype(np.float32)
        w = dis[:, None] * (xp @ np.asarray(W_down[lev], np.float32))
        CW, RW = k // 2, k // 4
        nc = ncB0 if lev == 0 else ncB1
        maps = []
        if lev == 0:
            aw = _pow2_for(np.abs(w).max(), 128.0)
        else:
            aw = _pow2_for(np.abs(w).max(), 8192.0)
        for c in range(NCORES):
            i, j = c // 2, c % 2
            m = {"Rc": _pack(np.ascontiguousarray(R[:, j * CW:(j + 1) * CW]), NP8),
                 "LrT": _pack(np.ascontiguousarray(L[i * RW:(i + 1) * RW].T), NP8)}
            wj = w[j * CW:(j + 1) * CW]
            if lev == 0:
                h8, l8 = _split8(wj, aw)
                m["w0"], m["w1"] = _pack(h8, NP8), _pack(l8, NP8)
            else:
                m["w0"] = _pack((wj * aw).astype(np.float32), NP16)
            maps.append(m)
        res = _run(nc, maps)
        # assemble M [k, k] and reduce P partials
        M = np.empty((k, k), np.float32)
        xnew = np.empty((k, C), np.float32)
        for i in range(4):
            Pi = (_unpack(res[2 * i]["Po"].astype(np.float32), RW // 128, C)
                  + _unpack(res[2 * i + 1]["Po"].astype(np.float32),
                            RW // 128, C)) / aw
            sl = slice(i * RW, (i + 1) * RW)
            xnew[sl] = np.maximum(
                dis[sl, None] * (Pi + (2.0 - diagM[sl, None]) * w[sl]), 0.0)
            for j in range(2):
                Xp = _unpack(res[2 * i + j]["XT"].astype(np.float32),
                             CW // 128, RW)
                M[sl, j * CW:(j + 1) * CW] = Xp.T
        if lev == 0:
            assert M.max() <= 15
        else:
            assert M.max() <= 255
        Ms.append(M)
        Bh = M - np.diag(np.diag(M)) + np.eye(k, dtype=np.float32)
        xs.append(xnew)
        xcur = xnew
        n = k

    # ---- level 2 (K4a): factored, no M3
    lev = 2
    p = np.asarray(p_pool[lev], np.float32)
    score = (xcur @ p) / np.linalg.norm(p)
    k = n // 2
    perm = np.argsort(-score, kind="stable")[:k]
    sv = score[perm]
    perms.append(perm)
    L3 = Bh[perm, :]
    R3 = Bh[:, perm]
    assert Bh.max() <= 255
    diagM3 = np.einsum('ak,ka->a', L3, R3, optimize=True).astype(np.float32)
    r = R3.sum(1, dtype=np.float64)
    deg = (L3 @ r.astype(np.float32)).astype(np.float64) - diagM3 + 2.0
    dis3 = _mk_dis(deg.astype(np.float32))
    diss.append(dis3)
    xp = (xcur[perm] * np.tanh(sv)[:, None]).astype(np.float32)
    w3 = dis3[:, None] * (xp @ np.asarray(W_down[2], np.float32))
    aw3 = _pow2_for(np.abs(w3).max(), 8192.0)
    R3_img = _pack(R3, NPBF)
    w3_img = _pack((w3 * aw3).astype(np.float32), NP16)
    ident128 = np.eye(128, dtype=np.float32).astype(NPBF)
    maps = [{"R3": R3_img, "w3": w3_img, "ident": ident128,
             "L3cT": _pack(np.ascontiguousarray(L3[c * 64:(c + 1) * 64].T),
                           NPBF)}
            for c in range(NCORES)]
    res = _run(ncC, maps)
    P3 = np.concatenate([res[c]["xo"] for c in range(NCORES)], 0) / aw3
    x_d2 = np.maximum(dis3[:, None] * (P3 + (2.0 - diagM3[:, None]) * w3), 0.0)

    # ---- K4b
    x_d0, x_d1 = xs[1], xs[2]
    dis1, dis2 = diss[1], diss[2]
    M1, M2 = Ms
    M1p = M1 - np.diag(np.diag(M1)) + 2.0 * np.eye(2048, dtype=np.float32)
    M2p = M2 - np.diag(np.diag(M2)) + 2.0 * np.eye(1024, dtype=np.float32)
    assert M1p.max() <= 15
    up = np.zeros_like(x_d1)
    up[perms[2]] = x_d2
    xa1 = x_d1 + up
    w_u = dis2[:, None] * (xa1 @ np.asarray(W_up[0], np.float32))
    w1 = dis1[:, None] * x_d0
    au = _pow2_for(np.abs(w_u).max(), 8192.0)
    sbase = (dis1[perms[1]] * dis2).astype(np.float32)
    boundX = float(sbase.max() * np.abs(M2p).sum(1).max() * np.abs(w_u).max())
    ac = min(_pow2_for(np.abs(w1).max(), 8192.0), _pow2_for(boundX, 8192.0))
    sp = sbase * (ac / au)
    M2pT_img = _pack(np.ascontiguousarray(M2p.T), NPBF)
    wu_img = _pack((w_u * au).astype(np.float32), NP16)
    w1_img = _pack((w1 * ac).astype(np.float32), NP16)
    Wu1_img = _pack(np.asarray(W_up[1], np.float32), NP16)
    sv_img = np.ascontiguousarray(sp.reshape(8, 128).T.astype(np.float32))
    ident = np.eye(128, dtype=np.float32).astype(NPBF)
    maps = []
    for c in range(NCORES):
        sl = slice(c * 256, (c + 1) * 256)
        Kst = np.vstack([M1p[sl].T, M1p[sl][:, perms[1]].T])
        d1 = (dis1[sl] / ac).astype(np.float32)
        maps.append({
            "M2pT": M2pT_img, "wu": wu_img, "w1": w1_img, "Wu1": Wu1_img,
            "Kst": _pack(np.ascontiguousarray(Kst), NP8),
            "ident": ident, "svec": sv_img,
            "d1vec": np.ascontiguousarray(d1.reshape(2, 128).T)})
    res = _run(ncD, maps)
    xU2 = np.concatenate([_unpack(res[c]["xo"], 2, C) for c in range(NCORES)], 0)

    # ---- K4c (module A again)
    up0 = np.zeros_like(x0)
    up0[perms[0]] = xU2
    g = (x0 + up0) @ np.asarray(W_final, np.float32)
    z2 = dis0[:, None] * g
    a2 = _pow2_for(np.abs(z2).max(), 128.0)
    z2h, z2l = _split8(z2, a2)
    z2h_img, z2l_img = _pack(z2h, NP8), _pack(z2l, NP8)
    maps = [{"AT": AT_imgs[c], "zh": z2h_img, "zl": z2l_img}
            for c in range(NCORES)]
    res = _run(ncA, maps)
    out = np.concatenate([_unpack(res[c]["po"], 4, C) for c in range(NCORES)], 0)
    out = dis0[:, None] * (out / a2) + 2.0 * dis0[:, None] ** 2 * g
    return out.astype(np.float32)


# revision 23
# speedup vs baseline: 1.0045x; 1.0045x over previous
"""GraphUNet (N=4096, E=65536, C=256, depth 3, ratio 0.5) on 8 trn2 NeuronCores.

Five compiled modules, six launches. Device does all adjacency matmuls
(A@x SpMMs and the dense pooled A@A products); host does O(n^2) prep,
top-k, permutation gathers, CxC weight folds, and scaling-vector algebra
(all folded out of the device programs).

  A  (K1+K4c) row-sharded N0-apply: psum = A0[rows] @ z, z host-split into
     two scaled fp8 halves (DoubleRow), raw f32 psums shipped; host applies
     dis scalings + 2*dis^2 diag term.
  B0 (K2) level-1: (4 row x 2 col)-grid M1 = L@R in fp8 DR; M^T col-blocks
     shipped fp8 (ints, exact); partial GCN P = X'^T @ w (fp8 DR) shipped
     f32; host reduces the 2 k-partials, applies dis/diag corrections+relu.
  B1 (K3) level-2: same at half size; M2 entries >16 so X' ships bf16 and
     the P-chain rhs is fp16.
  C  (K4a) level-3 factored GCN (no M3 materialization): u = R3 @ w3,
     x_rows = L3[rows] @ u; bf16/fp16.
  D  (K4b) both up-GCNs fused: xU1 = relu-scale(M2p^T-chain) written
     straight into the combined rhs tile; v2 = Kst^T @ [w1; xU1s] with the
     unpool-scatter folded into host-gathered Kst = [M1p; M1p[:,p2]]^T fp8;
     transpose + Wu1 matmul + relu on device.

All device inputs are host-packed [128, X] contiguous SBUF images (full
DMA bandwidth, no sub-512B descriptor penalty). All fp8/fp16 payloads are
pre-scaled by power-of-2 to dodge fp8's 2^-10 subnormal floor; scales are
folded into host-side post-processing (everything downstream is linear,
and relu commutes with positive scales).
"""

import numpy as np
import ml_dtypes

from contextlib import ExitStack

import concourse.bass as bass
import concourse.mybir as mybir
import concourse.tile as tile
from concourse import bacc
from concourse.bass_utils import run_bass_kernel_spmd

NCORES = 8
C = 256
F32 = mybir.dt.float32
F16 = mybir.dt.float16
BF16 = mybir.dt.bfloat16
FP8 = mybir.dt.float8e4

NP8 = ml_dtypes.float8_e4m3fn
NP16 = np.float16
NPBF = ml_dtypes.bfloat16

_TRACE = {"on": False, "results": [], "ncs": []}


# ------------------------------------------------------------- host helpers
def _pack(arr, np_dt):
    """[K, F] -> [128, (K//128)*F] image; k-tile o holds rows o*128..o*128+127."""
    K, F = arr.shape
    KT = K // 128
    return np.ascontiguousarray(
        arr.reshape(KT, 128, F).transpose(1, 0, 2).reshape(128, KT * F)
    ).astype(np_dt)


def _unpack(img, MO, F):
    """[128, MO*F] -> [MO*128, F] (inverse of _pack on the output side)."""
    return np.ascontiguousarray(
        img.reshape(128, MO, F).transpose(1, 0, 2).reshape(MO * 128, F))


def _pow2_for(m, target):
    m = float(m)
    return 1.0 if m <= 0 else float(2.0 ** np.floor(np.log2(target / m)))


def _split8(x, alpha):
    v = (x * alpha).astype(np.float32)
    h = v.astype(NP8)
    lo = (v - h.astype(np.float32)).astype(NP8)
    return h, lo


def _mk_dis(deg):
    return (1.0 / np.sqrt(np.maximum(deg, 1e-12))).astype(np.float32)


# ----------------------------------------------------------- device helpers
def _in_img(nc, name, KT, F, dt):
    return nc.dram_tensor(name, [128, KT * F], dt, kind="ExternalInput")


def _ld_chunks(nc, pool, dram, KT, F, tag, chunks):
    """Allocate [128, KT, F] tile; return (tile, chunk-issue fns).
    chunks: list of kt-counts per dma, or an int chunk size."""
    t = pool.tile([128, KT, F], dram.dtype, tag=tag, name=tag)
    r = dram.ap().rearrange("p (o f) -> p o f", f=F)
    if isinstance(chunks, int):
        chunks = [chunks] * ((KT + chunks - 1) // chunks)
    fns, k = [], 0
    for ck in chunks:
        k0, k1 = k, min(KT, k + ck)
        k = k1
        fns.append(lambda k0=k0, k1=k1: nc.sync.dma_start(
            t[:, k0:k1, :], r[:, k0:k1, :]))
        if k >= KT:
            break
    return t, fns


def _interleave(*fn_lists):
    n = max(len(f) for f in fn_lists)
    for i in range(n):
        for fns in fn_lists:
            if i < len(fns):
                fns[i]()


def _spread_copy(nc, idx, dst, src):
    if idx % 2 == 0:
        nc.scalar.copy(dst, src)
    else:
        nc.vector.tensor_copy(dst, src)


def _mm_ktouter(nc, ps, lhsT, rhs_list, M, NF, dr, tagp, consumer,
                stagger=False):
    """kt-outer accumulation: psums for all M//128 row-blocks live at once.
    lhsT [128, KT, M]; each rhs [128, KT, NF]. dr: fp8 DoubleRow.
    stagger: issue the last k-step mo-major with the consumer interleaved, so
    psum copies start as soon as each row-block's accumulation closes."""
    n_mo = M // 128
    KT = lhsT.shape[1]
    pss = [ps.tile([128, NF], F32, tag=f"{tagp}{m}", name=f"{tagp}{m}")
           for m in range(n_mo)]

    def mm(mo, k, ci, start, stop):
        if dr:
            nc.tensor.matmul(
                pss[mo][:],
                lhsT[:, 2 * k:2 * k + 2, mo * 128:(mo + 1) * 128],
                rhs_list[ci][:, 2 * k:2 * k + 2, :],
                start=start, stop=stop,
                perf_mode=mybir.MatmulPerfMode.DoubleRow)
        else:
            nc.tensor.matmul(
                pss[mo][:], lhsT[:, k, mo * 128:(mo + 1) * 128],
                rhs_list[ci][:, k, :], start=start, stop=stop)

    nch = len(rhs_list)
    KS = (KT // 2) if dr else KT
    nk_main = KS - 1 if (stagger and KS > 1) else KS
    step = 0
    for k in range(nk_main):
        for ci in range(nch):
            step += 1
            for mo in range(n_mo):
                mm(mo, k, ci, step == 1, step == KS * nch)
    if nk_main < KS:
        for mo in range(n_mo):
            for ci in range(nch):
                mm(mo, KS - 1, ci, False, ci == nch - 1)
            consumer(mo, pss[mo])
    else:
        for mo in range(n_mo):
            consumer(mo, pss[mo])


def _new_nc():
    return bacc.Bacc("TRN2", target_bir_lowering=False, debug=False,
                     num_devices=NCORES)


def _run(nc, in_maps):
    res = run_bass_kernel_spmd(nc, in_maps, list(range(NCORES)),
                               trace=_TRACE["on"])
    _TRACE["ncs"].append(nc)
    if _TRACE["on"]:
        _TRACE["results"].append(res)
    return res.results


# --------------------------------------------------------------- module A
def build_A():
    """psum[rows_c] = A0[rows_c] @ (zh + zl); rows_c = 512-row slab."""
    nc = _new_nc()
    KT, RW = 32, 512
    AT = _in_img(nc, "AT", KT, RW, FP8)
    zh = _in_img(nc, "zh", KT, C, FP8)
    zl = _in_img(nc, "zl", KT, C, FP8)
    po = nc.dram_tensor("po", [128, (RW // 128) * C], F32, kind="ExternalOutput")
    with tile.TileContext(nc) as tc:
        ctx = ExitStack()
        sb = ctx.enter_context(tc.tile_pool(name="sb", bufs=1))
        ps = ctx.enter_context(tc.tile_pool(name="ps", bufs=1, space="PSUM"))
        AT_sb, a_fns = _ld_chunks(nc, sb, AT, KT, RW, "AT", [6, 6, 6, 6, 4, 2, 2])
        zh_sb, h_fns = _ld_chunks(nc, sb, zh, KT, C, "zh", 8)
        zl_sb, l_fns = _ld_chunks(nc, sb, zl, KT, C, "zl", 8)
        _interleave(a_fns, h_fns, l_fns)
        o_sb = sb.tile([128, RW // 128, C], F32, tag="o", name="o")
        ro = po.ap().rearrange("p (o f) -> p o f", f=C)

        def fin(mo, p):
            _spread_copy(nc, mo, o_sb[:, mo, :], p[:])
            if mo % 2 == 1:
                nc.sync.dma_start(ro[:, mo - 1:mo + 1, :],
                                  o_sb[:, mo - 1:mo + 1, :])

        _mm_ktouter(nc, ps, AT_sb, [zh_sb, zl_sb], RW, C, True, "mp", fin,
                    stagger=True)
        ctx.close()
    nc.compile()
    return nc


# --------------------------------------------------------------- module B
def build_B(NPREV, NK, xdt, wsplit):
    """(4 rows x 2 cols) grid core: X' = M^T[cols_j, rows_i] (fp8 DR chain),
    P = X'^T @ w[cols_j] partial GCN. wsplit=2 -> two fp8 rhs (DR);
    wsplit=1 -> one fp16 rhs."""
    nc = _new_nc()
    KT = NPREV // 128
    CW, RW = NK // 2, NK // 4
    MOX, MOP = CW // 128, RW // 128
    Rc = _in_img(nc, "Rc", KT, CW, FP8)
    LrT = _in_img(nc, "LrT", KT, RW, FP8)
    wdt = FP8 if wsplit == 2 else F16
    ws = [_in_img(nc, f"w{i}", MOX, C, wdt) for i in range(wsplit)]
    XT = nc.dram_tensor("XT", [128, MOX * RW], xdt, kind="ExternalOutput")
    Po = nc.dram_tensor("Po", [128, MOP * C], BF16, kind="ExternalOutput")
    rck = [6, 6, 6, 6, 4, 2, 2] if KT == 32 else [4, 4, 4, 2, 2]
    with tile.TileContext(nc) as tc:
        ctx = ExitStack()
        sb = ctx.enter_context(tc.tile_pool(name="sb", bufs=1))
        ps = ctx.enter_context(tc.tile_pool(name="ps", bufs=1, space="PSUM"))
        Rc_sb, r_fns = _ld_chunks(nc, sb, Rc, KT, CW, "Rc", rck)
        LrT_sb, l_fns = _ld_chunks(nc, sb, LrT, KT, RW, "LrT", rck)
        w_sbs, w_fns = [], []
        for i, w in enumerate(ws):
            t, fns = _ld_chunks(nc, sb, w, MOX, C, f"w{i}", MOX)
            w_sbs.append(t)
            w_fns.append(fns)
        _interleave(r_fns, l_fns, *w_fns)
        X_sb = sb.tile([128, MOX, RW], xdt, tag="X", name="X")
        rx = XT.ap().rearrange("p (o f) -> p o f", f=RW)

        def xfin(mo, p):
            _spread_copy(nc, mo, X_sb[:, mo, :], p[:])
            if mo == MOX // 2 - 1 or mo == MOX - 1:
                nc.sync.dma_start(rx[:, mo - MOX // 2 + 1:mo + 1, :],
                                  X_sb[:, mo - MOX // 2 + 1:mo + 1, :])

        _mm_ktouter(nc, ps, Rc_sb, [LrT_sb], CW, RW, True, "mp", xfin,
                    stagger=True)
        P_sb = sb.tile([128, MOP, C], BF16, tag="P", name="P")

        def pfin(mo, p):
            _spread_copy(nc, mo + 1, P_sb[:, mo, :], p[:])

        _mm_ktouter(nc, ps, X_sb, w_sbs, RW, C, wsplit == 2, "mp", pfin,
                    stagger=True)
        nc.sync.dma_start(Po.ap(), P_sb[:].rearrange("p o f -> p (o f)"))
        ctx.close()
    nc.compile()
    return nc


# --------------------------------------------------------------- module C
def build_C():
    """M3c = L3[rows_c] @ R3 (rides the R3 stream), then transpose and
    x[rows_c] = M3c @ w3; 64 rows/core."""
    nc = _new_nc()
    R3 = _in_img(nc, "R3", 8, 512, BF16)
    w3 = _in_img(nc, "w3", 4, C, F16)
    L3cT = _in_img(nc, "L3cT", 8, 64, BF16)
    ident = nc.dram_tensor("ident", [128, 128], BF16, kind="ExternalInput")
    xo = nc.dram_tensor("xo", [64, C], F32, kind="ExternalOutput")
    with tile.TileContext(nc) as tc:
        ctx = ExitStack()
        sb = ctx.enter_context(tc.tile_pool(name="sb", bufs=1))
        ps = ctx.enter_context(tc.tile_pool(name="ps", bufs=1, space="PSUM"))
        L3_sb, l_fns = _ld_chunks(nc, sb, L3cT, 8, 64, "L3cT", 8)
        id_sb = sb.tile([128, 128], BF16, tag="id", name="id")
        R3_sb, r_fns = _ld_chunks(nc, sb, R3, 8, 512, "R3", [2, 2, 2, 1, 1])
        w3_sb, w_fns = _ld_chunks(nc, sb, w3, 4, C, "w3", 4)
        l_fns[0]()
        r_fns[0]()
        nc.sync.dma_start(id_sb[:], ident.ap())
        _interleave(r_fns[1:], w_fns)
        # M3c = L3c @ R3  [64, 512], kt-outer over the R3 stream
        pm = ps.tile([128, 512], F32, tag="pm", name="pm")
        for kt in range(8):
            nc.tensor.matmul(pm[:64, :], L3_sb[:, kt, :], R3_sb[:, kt, :],
                             start=(kt == 0), stop=(kt == 7))
        m3 = sb.tile([128, 512], BF16, tag="m3", name="m3")
        nc.scalar.copy(m3[:64, :], pm[:64, :])
        m3T = sb.tile([128, 4, 64], BF16, tag="m3T", name="m3T")
        for cc in range(4):
            pt = ps.tile([128, 64], BF16, tag=f"pt{cc % 2}", name="pt")
            nc.tensor.transpose(pt[:, :], m3[:64, cc * 128:(cc + 1) * 128],
                                id_sb[:64, :64])
            _spread_copy(nc, cc, m3T[:, cc, :], pt[:, :])
        px = ps.tile([128, C], F32, tag="px", name="px")
        for kt in range(4):
            nc.tensor.matmul(px[:64, :], m3T[:, kt, :], w3_sb[:, kt, :],
                             start=(kt == 0), stop=(kt == 3))
        o_sb = sb.tile([128, C], F32, tag="o", name="o")
        nc.scalar.copy(o_sb[:64, :], px[:64, :])
        nc.sync.dma_start(xo.ap(), o_sb[:64, :])
        ctx.close()
    nc.compile()
    return nc


# --------------------------------------------------------------- module D
def build_D():
    """xU1 = relu-scale(M2p-chain) -> rhs tile; v2 = Kst^T @ [w1; xU1s];
    xU2[rows_c] = relu((dis1-scaled v2) @ Wu1). 256 v2-rows per core."""
    nc = _new_nc()
    M2pT = _in_img(nc, "M2pT", 8, 1024, BF16)
    wu = _in_img(nc, "wu", 8, C, F16)
    Kst = _in_img(nc, "Kst", 24, C, FP8)
    w1 = _in_img(nc, "w1", 16, C, F16)
    Wu1 = _in_img(nc, "Wu1", 2, C, F16)
    ident = nc.dram_tensor("ident", [128, 128], BF16, kind="ExternalInput")
    svec = nc.dram_tensor("svec", [128, 8], F32, kind="ExternalInput")
    d1vec = nc.dram_tensor("d1vec", [128, 2], F32, kind="ExternalInput")
    xo = nc.dram_tensor("xo", [128, 2 * C], F32, kind="ExternalOutput")
    with tile.TileContext(nc) as tc:
        ctx = ExitStack()
        sb = ctx.enter_context(tc.tile_pool(name="sb", bufs=1))
        ps = ctx.enter_context(tc.tile_pool(name="ps", bufs=1, space="PSUM"))
        M2pT_sb, m_fns = _ld_chunks(nc, sb, M2pT, 8, 1024, "M2pT",
                                    [1, 2, 2, 1, 1, 1])
        wu_sb, wu_fns = _ld_chunks(nc, sb, wu, 8, C, "wu", [2, 3, 3])
        Kst_sb, k_fns = _ld_chunks(nc, sb, Kst, 24, C, "Kst", 6)
        Wu1_sb, wf_fns = _ld_chunks(nc, sb, Wu1, 2, C, "Wu1", 2)
        id_sb = sb.tile([128, 128], BF16, tag="id", name="id")
        sv_sb = sb.tile([128, 8], F32, tag="sv", name="sv")
        d1_sb = sb.tile([128, 2], F32, tag="d1", name="d1")
        rhs_sb = sb.tile([128, 24, C], F16, tag="rhs", name="rhs")

        def aux():
            nc.sync.dma_start(id_sb[:], ident.ap())
            nc.sync.dma_start(sv_sb[:], svec.ap())
            nc.sync.dma_start(d1_sb[:], d1vec.ap())
        r1 = w1.ap().rearrange("p (o f) -> p o f", f=C)
        w1_fns = [lambda k0=k0: nc.sync.dma_start(
            rhs_sb[:, k0:k0 + 4, :], r1[:, k0:k0 + 4, :])
            for k0 in range(0, 16, 4)]
        # M2pT+wu first (xU1 rides them), then Kst/w1 interleaved 1:1 so
        # each v2 k-step's lhsT and rhs chunks land together.
        aux()
        _interleave(m_fns, wu_fns)
        _interleave(k_fns, w1_fns, wf_fns)

        # xU1s written straight into rhs tile k-tiles 16..23; spread the
        # relu+scale over Act and DVE so the handoff to v2 isn't serial
        def xufin(mo, p):
            if mo % 2 == 0:
                nc.scalar.activation(rhs_sb[:, 16 + mo, :], p[:],
                                     mybir.ActivationFunctionType.Relu,
                                     scale=sv_sb[:, mo:mo + 1])
            else:
                nc.vector.tensor_scalar(rhs_sb[:, 16 + mo, :], p[:],
                                        sv_sb[:, mo:mo + 1], 0.0,
                                        mybir.AluOpType.mult,
                                        mybir.AluOpType.max)

        _mm_ktouter(nc, ps, M2pT_sb, [wu_sb], 1024, C, False, "mp", xufin,
                    stagger=True)

        v2b = sb.tile([128, 2, C], BF16, tag="v2b", name="v2b")

        def vfin(mo, p):
            if mo % 2 == 0:
                nc.scalar.activation(v2b[:, mo, :], p[:],
                                     mybir.ActivationFunctionType.Copy,
                                     scale=d1_sb[:, mo:mo + 1])
            else:
                nc.vector.tensor_scalar_mul(v2b[:, mo, :], p[:],
                                            d1_sb[:, mo:mo + 1])

        _mm_ktouter(nc, ps, Kst_sb, [rhs_sb], 256, C, False, "mp", vfin,
                    stagger=True)
        v2T = sb.tile([128, 2, C], BF16, tag="v2T", name="v2T")
        for mo in range(2):
            for cc in range(2):
                pst = ps.tile([128, 128], BF16, tag="mp4", name="pt")
                nc.tensor.transpose(pst[:], v2b[:, mo, cc * 128:(cc + 1) * 128],
                                    id_sb[:])
                _spread_copy(nc, mo, v2T[:, cc, mo * 128:(mo + 1) * 128],
                             pst[:])
        o_sb = sb.tile([128, 2, C], F32, tag="o", name="o")
        ro = xo.ap().rearrange("p (o f) -> p o f", f=C)

        def ofin(mo, p):
            if mo % 2 == 0:
                nc.scalar.activation(o_sb[:, mo, :], p[:],
                                     mybir.ActivationFunctionType.Relu)
            else:
                nc.vector.tensor_scalar_max(o_sb[:, mo, :], p[:], 0.0)
            nc.sync.dma_start(ro[:, mo, :], o_sb[:, mo, :])

        _mm_ktouter(nc, ps, v2T, [Wu1_sb], 256, C, False, "mp", ofin,
                    stagger=True)
        ctx.close()
    nc.compile()
    return nc


# =================================================================== host
def kernel(x, edge_index, W_init, b_init, W_down, b_down, p_pool,
           W_up, b_up, W_final, b_final):
    x = np.asarray(x, np.float32)
    N = x.shape[0]

    A0 = np.zeros((N, N), np.float32)
    np.add.at(A0, (np.asarray(edge_index[0]), np.asarray(edge_index[1])), 1.0)
    assert A0.max() <= 15
    dis0 = _mk_dis(A0.sum(1) + 2.0)
    y0 = x @ np.asarray(W_init, np.float32)
    z = dis0[:, None] * y0

    ncA = build_A()
    ncB0 = build_B(4096, 2048, FP8, 2)
    ncB1 = build_B(2048, 1024, BF16, 1)
    ncC = build_C()
    ncD = build_D()

    # per-core A0 row-slab lhsT images (shared by K1 and K4c)
    AT_imgs = [_pack(np.ascontiguousarray(A0[c * 512:(c + 1) * 512].T), NP8)
               for c in range(NCORES)]

    # ---- K1
    az = _pow2_for(np.abs(z).max(), 128.0)
    zh, zl = _split8(z, az)
    zh_img, zl_img = _pack(zh, NP8), _pack(zl, NP8)
    maps = [{"AT": AT_imgs[c], "zh": zh_img, "zl": zl_img}
            for c in range(NCORES)]
    res = _run(ncA, maps)
    x0 = np.concatenate([_unpack(res[c]["po"], 4, C) for c in range(NCORES)], 0)
    x0 = dis0[:, None] * (x0 / az) + 2.0 * dis0[:, None] ** 2 * y0

    # ---- down levels 0,1 (K2, K3)
    Bh = A0 + np.eye(N, dtype=np.float32)
    xcur = x0
    n = N
    xs = [x0]
    Ms, perms, diss = [], [], [dis0]
    for lev in range(2):
        p = np.asarray(p_pool[lev], np.float32)
        score = (xcur @ p) / np.linalg.norm(p)
        k = n // 2
        perm = np.argsort(-score, kind="stable")[:k]
        sv = score[perm]
        perms.append(perm)
        L = Bh[perm, :]
        R = Bh[:, perm]
        assert Bh.max() <= 15
        diagM = np.einsum('ak,ka->a', L, R, optimize=True).astype(np.float32)
        r = R.sum(1, dtype=np.float64)
        deg = (L @ r.astype(np.float32)).astype(np.float64) - diagM + 2.0
        dis = _mk_dis(deg.astype(np.float32))
        diss.append(dis)
        xp = (xcur[perm] * np.tanh(sv)[:, None]).astype(np.float32)
        w = dis[:, None] * (xp @ np.asarray(W_down[lev], np.float32))
        CW, RW = k // 2, k // 4
        nc = ncB0 if lev == 0 else ncB1
        maps = []
        if lev == 0:
            aw = _pow2_for(np.abs(w).max(), 128.0)
        else:
            aw = _pow2_for(np.abs(w).max(), 8192.0)
        for c in range(NCORES):
            i, j = c // 2, c % 2
            m = {"Rc": _pack(np.ascontiguousarray(R[:, j * CW:(j + 1) * CW]), NP8),
                 "LrT": _pack(np.ascontiguousarray(L[i * RW:(i + 1) * RW].T), NP8)}
            wj = w[j * CW:(j + 1) * CW]
            if lev == 0:
                h8, l8 = _split8(wj, aw)
                m["w0"], m["w1"] = _pack(h8, NP8), _pack(l8, NP8)
            else:
                m["w0"] = _pack((wj * aw).astype(np.float32), NP16)
            maps.append(m)
        res = _run(nc, maps)
        # assemble M [k, k] and reduce P partials
        M = np.empty((k, k), np.float32)
        xnew = np.empty((k, C), np.float32)
        for i in range(4):
            Pi = (_unpack(res[2 * i]["Po"].astype(np.float32), RW // 128, C)
                  + _unpack(res[2 * i + 1]["Po"].astype(np.float32),
                            RW // 128, C)) / aw
            sl = slice(i * RW, (i + 1) * RW)
            xnew[sl] = np.maximum(
                dis[sl, None] * (Pi + (2.0 - diagM[sl, None]) * w[sl]), 0.0)
            for j in range(2):
                Xp = _unpack(res[2 * i + j]["XT"].astype(np.float32),
                             CW // 128, RW)
                M[sl, j * CW:(j + 1) * CW] = Xp.T
        if lev == 0:
            assert M.max() <= 15
        else:
            assert M.max() <= 255
        Ms.append(M)
        Bh = M - np.diag(np.diag(M)) + np.eye(k, dtype=np.float32)
        xs.append(xnew)
        xcur = xnew
        n = k

    # ---- level 2 (K4a): factored, no M3
    lev = 2
    p = np.asarray(p_pool[lev], np.float32)
    score = (xcur @ p) / np.linalg.norm(p)
    k = n // 2
    perm = np.argsort(-score, kind="stable")[:k]
    sv = score[perm]
    perms.append(perm)
    L3 = Bh[perm, :]
    R3 = Bh[:, perm]
    assert Bh.max() <= 255
    diagM3 = np.einsum('ak,ka->a', L3, R3, optimize=True).astype(np.float32)
    r = R3.sum(1, dtype=np.float64)
    deg = (L3 @ r.astype(np.float32)).astype(np.float64) - diagM3 + 2.0
    dis3 = _mk_dis(deg.astype(np.float32))
    diss.append(dis3)
    xp = (xcur[perm] * np.tanh(sv)[:, None]).astype(np.float32)
    w3 = dis3[:, None] * (xp @ np.asarray(W_down[2], np.float32))
    aw3 = _pow2_for(np.abs(w3).max(), 8192.0)
    R3_img = _pack(R3, NPBF)
    w3_img = _pack((w3 * aw3).astype(np.float32), NP16)
    ident128 = np.eye(128, dtype=np.float32).astype(NPBF)
    maps = [{"R3": R3_img, "w3": w3_img, "ident": ident128,
             "L3cT": _pack(np.ascontiguousarray(L3[c * 64:(c + 1) * 64].T),
                           NPBF)}
            for c in range(NCORES)]
    res = _run(ncC, maps)
    P3 = np.concatenate([res[c]["xo"] for c in range(NCORES)], 0) / aw3
    x_d2 = np.maximum(dis3[:, None] * (P3 + (2.0 - diagM3[:, None]) * w3), 0.0)

    # ---- K4b
    x_d0, x_d1 = xs[1], xs[2]
    dis1, dis2 = diss[1], diss[2]
    M1, M2 = Ms
    M1p = M1 - np.diag(np.diag(M1)) + 2.0 * np.eye(2048, dtype=np.float32)
    M2p = M2 - np.diag(np.diag(M2)) + 2.0 * np.eye(1024, dtype=np.float32)
    assert M1p.max() <= 15
    up = np.zeros_like(x_d1)
    up[perms[2]] = x_d2
    xa1 = x_d1 + up
    w_u = dis2[:, None] * (xa1 @ np.asarray(W_up[0], np.float32))
    w1 = dis1[:, None] * x_d0
    au = _pow2_for(np.abs(w_u).max(), 8192.0)
    sbase = (dis1[perms[1]] * dis2).astype(np.float32)
    boundX = float(sbase.max() * np.abs(M2p).sum(1).max() * np.abs(w_u).max())
    ac = min(_pow2_for(np.abs(w1).max(), 8192.0), _pow2_for(boundX, 8192.0))
    sp = sbase * (ac / au)
    M2pT_img = _pack(np.ascontiguousarray(M2p.T), NPBF)
    wu_img = _pack((w_u * au).astype(np.float32), NP16)
    w1_img = _pack((w1 * ac).astype(np.float32), NP16)
    Wu1_img = _pack(np.asarray(W_up[1], np.float32), NP16)
    sv_img = np.ascontiguousarray(sp.reshape(8, 128).T.astype(np.float32))
    ident = np.eye(128, dtype=np.float32).astype(NPBF)
    maps = []
    for c in range(NCORES):
        sl = slice(c * 256, (c + 1) * 256)
        Kst = np.vstack([M1p[sl].T, M1p[sl][:, perms[1]].T])
        d1 = (dis1[sl] / ac).astype(np.float32)
        maps.append({
            "M2pT": M2pT_img, "wu": wu_img, "w1": w1_img, "Wu1": Wu1_img,
            "Kst": _pack(np.ascontiguousarray(Kst), NP8),
            "ident": ident, "svec": sv_img,
            "d1vec": np.ascontiguousarray(d1.reshape(2, 128).T)})
    res = _run(ncD, maps)
    xU2 = np.concatenate([_unpack(res[c]["xo"], 2, C) for c in range(NCORES)], 0)

    # ---- K4c (module A again)
    up0 = np.zeros_like(x0)
    up0[perms[0]] = xU2
    g = (x0 + up0) @ np.asarray(W_final, np.float32)
    z2 = dis0[:, None] * g
    a2 = _pow2_for(np.abs(z2).max(), 128.0)
    z2h, z2l = _split8(z2, a2)
    z2h_img, z2l_img = _pack(z2h, NP8), _pack(z2l, NP8)
    maps = [{"AT": AT_imgs[c], "zh": z2h_img, "zl": z2l_img}
            for c in range(NCORES)]
    res = _run(ncA, maps)
    out = np.concatenate([_unpack(res[c]["po"], 4, C) for c in range(NCORES)], 0)
    out = dis0[:, None] * (out / a2) + 2.0 * dis0[:, None] ** 2 * g
    return out.astype(np.float32)


# revision 29
# speedup vs baseline: 1.0106x; 1.0060x over previous
"""GraphUNet (N=4096, E=65536, C=256, depth 3, ratio 0.5) on 8 trn2 NeuronCores.

Five compiled modules, six launches. Device does all adjacency matmuls
(A@x SpMMs and the dense pooled A@A products); host does O(n^2) prep,
top-k, permutation gathers, CxC weight folds, and scaling-vector algebra
(all folded out of the device programs).

  A  (K1+K4c) row-sharded N0-apply: psum = A0[rows] @ z, z host-split into
     two scaled fp8 halves (DoubleRow), raw f32 psums shipped; host applies
     dis scalings + 2*dis^2 diag term.
  B0 (K2) level-1: (4 row x 2 col)-grid M1 = L@R in fp8 DR; M^T col-blocks
     shipped fp8 (ints, exact); partial GCN P = X'^T @ w (fp8 DR) shipped
     f32; host reduces the 2 k-partials, applies dis/diag corrections+relu.
  B1 (K3) level-2: same at half size; M2 entries >16 so X' ships bf16 and
     the P-chain rhs is fp16.
  C  (K4a) level-3 factored GCN (no M3 materialization): u = R3 @ w3,
     x_rows = L3[rows] @ u; bf16/fp16.
  D  (K4b) both up-GCNs fused: xU1 = relu-scale(M2p^T-chain) written
     straight into the combined rhs tile; v2 = Kst^T @ [w1; xU1s] with the
     unpool-scatter folded into host-gathered Kst = [M1p; M1p[:,p2]]^T fp8;
     transpose + Wu1 matmul + relu on device.

All device inputs are host-packed [128, X] contiguous SBUF images (full
DMA bandwidth, no sub-512B descriptor penalty). All fp8/fp16 payloads are
pre-scaled by power-of-2 to dodge fp8's 2^-10 subnormal floor; scales are
folded into host-side post-processing (everything downstream is linear,
and relu commutes with positive scales).
"""

import numpy as np
import ml_dtypes

from contextlib import ExitStack

import concourse.bass as bass
import concourse.mybir as mybir
import concourse.tile as tile
from concourse import bacc
from concourse.bass_utils import run_bass_kernel_spmd

NCORES = 8
C = 256
F32 = mybir.dt.float32
F16 = mybir.dt.float16
BF16 = mybir.dt.bfloat16
FP8 = mybir.dt.float8e4

NP8 = ml_dtypes.float8_e4m3fn
NP16 = np.float16
NPBF = ml_dtypes.bfloat16

_TRACE = {"on": False, "results": [], "ncs": []}


# ------------------------------------------------------------- host helpers
def _pack(arr, np_dt):
    """[K, F] -> [128, (K//128)*F] image; k-tile o holds rows o*128..o*128+127."""
    K, F = arr.shape
    KT = K // 128
    return np.ascontiguousarray(
        arr.reshape(KT, 128, F).transpose(1, 0, 2).reshape(128, KT * F)
    ).astype(np_dt)


def _unpack(img, MO, F):
    """[128, MO*F] -> [MO*128, F] (inverse of _pack on the output side)."""
    return np.ascontiguousarray(
        img.reshape(128, MO, F).transpose(1, 0, 2).reshape(MO * 128, F))


def _pow2_for(m, target):
    m = float(m)
    return 1.0 if m <= 0 else float(2.0 ** np.floor(np.log2(target / m)))


def _split8(x, alpha):
    v = (x * alpha).astype(np.float32)
    h = v.astype(NP8)
    lo = (v - h.astype(np.float32)).astype(NP8)
    return h, lo


def _mk_dis(deg):
    return (1.0 / np.sqrt(np.maximum(deg, 1e-12))).astype(np.float32)


# ----------------------------------------------------------- device helpers
def _in_img(nc, name, KT, F, dt):
    return nc.dram_tensor(name, [128, KT * F], dt, kind="ExternalInput")


def _ld_chunks(nc, pool, dram, KT, F, tag, chunks):
    """Allocate [128, KT, F] tile; return (tile, chunk-issue fns).
    chunks: list of kt-counts per dma, or an int chunk size."""
    t = pool.tile([128, KT, F], dram.dtype, tag=tag, name=tag)
    r = dram.ap().rearrange("p (o f) -> p o f", f=F)
    if isinstance(chunks, int):
        chunks = [chunks] * ((KT + chunks - 1) // chunks)
    fns, k = [], 0
    for ck in chunks:
        k0, k1 = k, min(KT, k + ck)
        k = k1
        fns.append(lambda k0=k0, k1=k1: nc.sync.dma_start(
            t[:, k0:k1, :], r[:, k0:k1, :]))
        if k >= KT:
            break
    return t, fns


def _interleave(*fn_lists):
    n = max(len(f) for f in fn_lists)
    for i in range(n):
        for fns in fn_lists:
            if i < len(fns):
                fns[i]()


def _spread_copy(nc, idx, dst, src):
    if idx % 2 == 0:
        nc.scalar.copy(dst, src)
    else:
        nc.vector.tensor_copy(dst, src)


def _mm_ktouter(nc, ps, lhsT, rhs_list, M, NF, dr, tagp, consumer,
                stagger=False):
    """kt-outer accumulation: psums for all M//128 row-blocks live at once.
    lhsT [128, KT, M]; each rhs [128, KT, NF]. dr: fp8 DoubleRow.
    stagger: issue the last k-step mo-major with the consumer interleaved, so
    psum copies start as soon as each row-block's accumulation closes."""
    n_mo = M // 128
    KT = lhsT.shape[1]
    pss = [ps.tile([128, NF], F32, tag=f"{tagp}{m}", name=f"{tagp}{m}")
           for m in range(n_mo)]

    def mm(mo, k, ci, start, stop):
        if dr:
            nc.tensor.matmul(
                pss[mo][:],
                lhsT[:, 2 * k:2 * k + 2, mo * 128:(mo + 1) * 128],
                rhs_list[ci][:, 2 * k:2 * k + 2, :],
                start=start, stop=stop,
                perf_mode=mybir.MatmulPerfMode.DoubleRow)
        else:
            nc.tensor.matmul(
                pss[mo][:], lhsT[:, k, mo * 128:(mo + 1) * 128],
                rhs_list[ci][:, k, :], start=start, stop=stop)

    nch = len(rhs_list)
    KS = (KT // 2) if dr else KT
    nk_main = KS - 1 if (stagger and KS > 1) else KS
    step = 0
    for k in range(nk_main):
        for ci in range(nch):
            step += 1
            for mo in range(n_mo):
                mm(mo, k, ci, step == 1, step == KS * nch)
    if nk_main < KS:
        for mo in range(n_mo):
            for ci in range(nch):
                mm(mo, KS - 1, ci, False, ci == nch - 1)
            consumer(mo, pss[mo])
    else:
        for mo in range(n_mo):
            consumer(mo, pss[mo])


def _new_nc():
    return bacc.Bacc("TRN2", target_bir_lowering=False, debug=False,
                     num_devices=NCORES)


def _run(nc, in_maps):
    res = run_bass_kernel_spmd(nc, in_maps, list(range(NCORES)),
                               trace=_TRACE["on"])
    _TRACE["ncs"].append(nc)
    if _TRACE["on"]:
        _TRACE["results"].append(res)
    return res.results


# --------------------------------------------------------------- module A
def build_A():
    """psum[rows_c] = A0[rows_c] @ (zh + zl); rows_c = 512-row slab."""
    nc = _new_nc()
    KT, RW = 32, 512
    AT = _in_img(nc, "AT", KT, RW, FP8)
    zh = _in_img(nc, "zh", KT, C, FP8)
    zl = _in_img(nc, "zl", KT, C, FP8)
    po = nc.dram_tensor("po", [128, (RW // 128) * C], F32, kind="ExternalOutput")
    with tile.TileContext(nc) as tc:
        ctx = ExitStack()
        sb = ctx.enter_context(tc.tile_pool(name="sb", bufs=1))
        ps = ctx.enter_context(tc.tile_pool(name="ps", bufs=1, space="PSUM"))
        AT_sb, a_fns = _ld_chunks(nc, sb, AT, KT, RW, "AT", [6, 6, 6, 6, 4, 2, 2])
        zh_sb, h_fns = _ld_chunks(nc, sb, zh, KT, C, "zh", 8)
        zl_sb, l_fns = _ld_chunks(nc, sb, zl, KT, C, "zl", 8)
        _interleave(a_fns, h_fns, l_fns)
        o_sb = sb.tile([128, RW // 128, C], F32, tag="o", name="o")
        ro = po.ap().rearrange("p (o f) -> p o f", f=C)

        def fin(mo, p):
            _spread_copy(nc, mo, o_sb[:, mo, :], p[:])
            if mo % 2 == 1:
                nc.sync.dma_start(ro[:, mo - 1:mo + 1, :],
                                  o_sb[:, mo - 1:mo + 1, :])

        _mm_ktouter(nc, ps, AT_sb, [zh_sb, zl_sb], RW, C, True, "mp", fin,
                    stagger=True)
        ctx.close()
    nc.compile()
    return nc


# --------------------------------------------------------------- module B
def build_B(NPREV, NK, xdt, wsplit):
    """(4 rows x 2 cols) grid core: X' = M^T[cols_j, rows_i] (fp8 DR chain),
    P = X'^T @ w[cols_j] partial GCN. wsplit=2 -> two fp8 rhs (DR);
    wsplit=1 -> one fp16 rhs."""
    nc = _new_nc()
    KT = NPREV // 128
    CW, RW = NK // 2, NK // 4
    MOX, MOP = CW // 128, RW // 128
    Rc = _in_img(nc, "Rc", KT, CW, FP8)
    LrT = _in_img(nc, "LrT", KT, RW, FP8)
    wdt = FP8 if wsplit == 2 else F16
    ws = [_in_img(nc, f"w{i}", MOX, C, wdt) for i in range(wsplit)]
    XT = nc.dram_tensor("XT", [128, MOX * RW], xdt, kind="ExternalOutput")
    Po = nc.dram_tensor("Po", [128, MOP * C], BF16, kind="ExternalOutput")
    rck = [6, 6, 6, 6, 4, 2, 2] if KT == 32 else [4, 4, 4, 2, 2]
    with tile.TileContext(nc) as tc:
        ctx = ExitStack()
        sb = ctx.enter_context(tc.tile_pool(name="sb", bufs=1))
        ps = ctx.enter_context(tc.tile_pool(name="ps", bufs=1, space="PSUM"))
        Rc_sb, r_fns = _ld_chunks(nc, sb, Rc, KT, CW, "Rc", rck)
        LrT_sb, l_fns = _ld_chunks(nc, sb, LrT, KT, RW, "LrT", rck)
        w_sbs, w_fns = [], []
        for i, w in enumerate(ws):
            t, fns = _ld_chunks(nc, sb, w, MOX, C, f"w{i}", MOX)
            w_sbs.append(t)
            w_fns.append(fns)
        _interleave(r_fns, l_fns, *w_fns)
        X_sb = sb.tile([128, MOX, RW], xdt, tag="X", name="X")
        rx = XT.ap().rearrange("p (o f) -> p o f", f=RW)

        def xfin(mo, p):
            _spread_copy(nc, mo, X_sb[:, mo, :], p[:])
            if mo == MOX // 2 - 1 or mo == MOX - 1:
                nc.sync.dma_start(rx[:, mo - MOX // 2 + 1:mo + 1, :],
                                  X_sb[:, mo - MOX // 2 + 1:mo + 1, :])

        _mm_ktouter(nc, ps, Rc_sb, [LrT_sb], CW, RW, True, "mp", xfin,
                    stagger=True)
        P_sb = sb.tile([128, MOP, C], BF16, tag="P", name="P")

        def pfin(mo, p):
            _spread_copy(nc, mo + 1, P_sb[:, mo, :], p[:])

        _mm_ktouter(nc, ps, X_sb, w_sbs, RW, C, wsplit == 2, "mp", pfin,
                    stagger=True)
        nc.sync.dma_start(Po.ap(), P_sb[:].rearrange("p o f -> p (o f)"))
        ctx.close()
    nc.compile()
    return nc


# --------------------------------------------------------------- module C
def build_C():
    """M3c = L3[rows_c] @ R3 (rides the R3 stream), then transpose and
    x[rows_c] = M3c @ w3; 64 rows/core."""
    nc = _new_nc()
    R3 = _in_img(nc, "R3", 8, 512, BF16)
    w3 = _in_img(nc, "w3", 4, C, F16)
    L3cT = _in_img(nc, "L3cT", 8, 64, BF16)
    ident = nc.dram_tensor("ident", [128, 128], BF16, kind="ExternalInput")
    xo = nc.dram_tensor("xo", [64, C], F32, kind="ExternalOutput")
    with tile.TileContext(nc) as tc:
        ctx = ExitStack()
        sb = ctx.enter_context(tc.tile_pool(name="sb", bufs=1))
        ps = ctx.enter_context(tc.tile_pool(name="ps", bufs=1, space="PSUM"))
        L3_sb, l_fns = _ld_chunks(nc, sb, L3cT, 8, 64, "L3cT", 8)
        id_sb = sb.tile([128, 128], BF16, tag="id", name="id")
        R3_sb, r_fns = _ld_chunks(nc, sb, R3, 8, 512, "R3", [2, 2, 2, 1, 1])
        w3_sb, w_fns = _ld_chunks(nc, sb, w3, 4, C, "w3", 4)
        l_fns[0]()
        r_fns[0]()
        nc.sync.dma_start(id_sb[:], ident.ap())
        _interleave(r_fns[1:], w_fns)
        # M3c = L3c @ R3  [64, 512], kt-outer over the R3 stream
        pm = ps.tile([128, 512], F32, tag="pm", name="pm")
        for kt in range(8):
            nc.tensor.matmul(pm[:64, :], L3_sb[:, kt, :], R3_sb[:, kt, :],
                             start=(kt == 0), stop=(kt == 7))
        m3 = sb.tile([128, 512], BF16, tag="m3", name="m3")
        nc.scalar.copy(m3[:64, :], pm[:64, :])
        m3T = sb.tile([128, 4, 64], BF16, tag="m3T", name="m3T")
        for cc in range(4):
            pt = ps.tile([128, 64], BF16, tag=f"pt{cc % 2}", name="pt")
            nc.tensor.transpose(pt[:, :], m3[:64, cc * 128:(cc + 1) * 128],
                                id_sb[:64, :64])
            _spread_copy(nc, cc, m3T[:, cc, :], pt[:, :])
        px = ps.tile([128, C], F32, tag="px", name="px")
        for kt in range(4):
            nc.tensor.matmul(px[:64, :], m3T[:, kt, :], w3_sb[:, kt, :],
                             start=(kt == 0), stop=(kt == 3))
        o_sb = sb.tile([128, C], F32, tag="o", name="o")
        nc.scalar.copy(o_sb[:64, :], px[:64, :])
        nc.sync.dma_start(xo.ap(), o_sb[:64, :])
        ctx.close()
    nc.compile()
    return nc


# --------------------------------------------------------------- module D
def build_D():
    """v2 w1-part first (fp8 DR, rides the Kst/w1 stream, parked in SBUF),
    then xU1 = relu-scale(M2p-chain) rides the M2pT stream, then the
    xU1s-part is added; xU2[rows_c] = relu((dis1-scaled v2) @ Wu1)."""
    nc = _new_nc()
    M2pT = _in_img(nc, "M2pT", 8, 1024, BF16)
    wu = _in_img(nc, "wu", 8, C, F16)
    Kst = _in_img(nc, "Kst", 24, C, FP8)
    w1h = _in_img(nc, "w1h", 16, C, FP8)
    w1l = _in_img(nc, "w1l", 16, C, FP8)
    Wu1 = _in_img(nc, "Wu1", 2, C, F16)
    ident = nc.dram_tensor("ident", [128, 128], BF16, kind="ExternalInput")
    svec = nc.dram_tensor("svec", [128, 8], F32, kind="ExternalInput")
    d1vec = nc.dram_tensor("d1vec", [128, 2], F32, kind="ExternalInput")
    xo = nc.dram_tensor("xo", [128, 2 * C], F32, kind="ExternalOutput")
    with tile.TileContext(nc) as tc:
        ctx = ExitStack()
        sb = ctx.enter_context(tc.tile_pool(name="sb", bufs=1))
        ps = ctx.enter_context(tc.tile_pool(name="ps", bufs=1, space="PSUM"))
        Kst_sb, k_fns = _ld_chunks(nc, sb, Kst, 24, C, "Kst", [8, 8, 8])
        w1h_sb, wh_fns = _ld_chunks(nc, sb, w1h, 16, C, "w1h", 8)
        w1l_sb, wl_fns = _ld_chunks(nc, sb, w1l, 16, C, "w1l", 8)
        M2pT_sb, m_fns = _ld_chunks(nc, sb, M2pT, 8, 1024, "M2pT",
                                    [1, 2, 2, 1, 1, 1])
        wu_sb, wu_fns = _ld_chunks(nc, sb, wu, 8, C, "wu", [2, 3, 3])
        Wu1_sb, wf_fns = _ld_chunks(nc, sb, Wu1, 2, C, "Wu1", 2)
        id_sb = sb.tile([128, 128], BF16, tag="id", name="id")
        sv_sb = sb.tile([128, 8], F32, tag="sv", name="sv")
        d1_sb = sb.tile([128, 2], F32, tag="d1", name="d1")
        rhs_sb = sb.tile([128, 8, C], F16, tag="rhs", name="rhs")

        nc.sync.dma_start(id_sb[:], ident.ap())
        nc.sync.dma_start(sv_sb[:], svec.ap())
        nc.sync.dma_start(d1_sb[:], d1vec.ap())
        _interleave(k_fns, wh_fns, wl_fns, wf_fns)
        _interleave(m_fns, wu_fns)

        # v2 part 1: Kst k-tiles 0..15 (fp8 DR over split w1) -> park in SBUF
        v2a = sb.tile([128, 2, C], F32, tag="v2a", name="v2a")
        vps = [ps.tile([128, C], F32, tag=f"mp{m}", name=f"v2p{m}")
               for m in range(2)]
        cnt = 0
        for kp in range(8):
            for rhs in (w1h_sb, w1l_sb):
                cnt += 1
                for mo in range(2):
                    nc.tensor.matmul(
                        vps[mo][:],
                        Kst_sb[:, 2 * kp:2 * kp + 2, mo * 128:(mo + 1) * 128],
                        rhs[:, 2 * kp:2 * kp + 2, :],
                        start=(cnt == 1), stop=(cnt == 16),
                        perf_mode=mybir.MatmulPerfMode.DoubleRow)
        nc.scalar.activation(v2a[:, 0, :], vps[0][:],
                             mybir.ActivationFunctionType.Copy,
                             scale=d1_sb[:, 0:1])
        nc.vector.tensor_scalar_mul(v2a[:, 1, :], vps[1][:], d1_sb[:, 1:2])

        # xU1s written into rhs tile k-tiles 0..7; spread relu+scale over
        # Act and DVE so the handoff to v2 part 2 isn't serial
        def xufin(mo, p):
            if mo % 2 == 0:
                nc.scalar.activation(rhs_sb[:, mo, :], p[:],
                                     mybir.ActivationFunctionType.Relu,
                                     scale=sv_sb[:, mo:mo + 1])
            else:
                nc.vector.tensor_scalar(rhs_sb[:, mo, :], p[:],
                                        sv_sb[:, mo:mo + 1], 0.0,
                                        mybir.AluOpType.mult,
                                        mybir.AluOpType.max)

        _mm_ktouter(nc, ps, M2pT_sb, [wu_sb], 1024, C, False, "mp", xufin,
                    stagger=True)

        # v2 part 2 (Kst k-tiles 16..23 vs xU1s) + parked part 1, scaled
        v2b = sb.tile([128, 2, C], BF16, tag="v2b", name="v2b")
        v2ps = [ps.tile([128, C], F32, tag=f"mp{m}", name=f"v2q{m}")
                for m in range(2)]
        for kt in range(8):
            for mo in range(2):
                nc.tensor.matmul(
                    v2ps[mo][:], Kst_sb[:, 16 + kt, mo * 128:(mo + 1) * 128],
                    rhs_sb[:, kt, :], start=(kt == 0), stop=(kt == 7))
        for mo in range(2):
            nc.vector.scalar_tensor_tensor(
                v2b[:, mo, :], v2ps[mo][:], d1_sb[:, mo:mo + 1],
                v2a[:, mo, :], mybir.AluOpType.mult, mybir.AluOpType.add)
        v2T = sb.tile([128, 2, C], BF16, tag="v2T", name="v2T")
        for mo in range(2):
            for cc in range(2):
                pst = ps.tile([128, 128], BF16, tag=f"mp{4 + (mo * 2 + cc) % 2}",
                              name="pt")
                nc.tensor.transpose(pst[:], v2b[:, mo, cc * 128:(cc + 1) * 128],
                                    id_sb[:])
                _spread_copy(nc, mo * 2 + cc, v2T[:, cc, mo * 128:(mo + 1) * 128],
                             pst[:])
        o_sb = sb.tile([128, 2, C], F32, tag="o", name="o")
        ro = xo.ap().rearrange("p (o f) -> p o f", f=C)

        def ofin(mo, p):
            if mo % 2 == 0:
                nc.scalar.activation(o_sb[:, mo, :], p[:],
                                     mybir.ActivationFunctionType.Relu)
            else:
                nc.vector.tensor_scalar_max(o_sb[:, mo, :], p[:], 0.0)
            nc.sync.dma_start(ro[:, mo, :], o_sb[:, mo, :])

        _mm_ktouter(nc, ps, v2T, [Wu1_sb], 256, C, False, "mp", ofin,
                    stagger=True)
        ctx.close()
    nc.compile()
    return nc


# =================================================================== host
def kernel(x, edge_index, W_init, b_init, W_down, b_down, p_pool,
           W_up, b_up, W_final, b_final):
    x = np.asarray(x, np.float32)
    N = x.shape[0]

    A0 = np.zeros((N, N), np.float32)
    np.add.at(A0, (np.asarray(edge_index[0]), np.asarray(edge_index[1])), 1.0)
    assert A0.max() <= 15
    dis0 = _mk_dis(A0.sum(1) + 2.0)
    y0 = x @ np.asarray(W_init, np.float32)
    z = dis0[:, None] * y0

    ncA = build_A()
    ncB0 = build_B(4096, 2048, FP8, 2)
    ncB1 = build_B(2048, 1024, BF16, 1)
    ncC = build_C()
    ncD = build_D()

    # per-core A0 row-slab lhsT images (shared by K1 and K4c)
    AT_imgs = [_pack(np.ascontiguousarray(A0[c * 512:(c + 1) * 512].T), NP8)
               for c in range(NCORES)]

    # ---- K1
    az = _pow2_for(np.abs(z).max(), 128.0)
    zh, zl = _split8(z, az)
    zh_img, zl_img = _pack(zh, NP8), _pack(zl, NP8)
    maps = [{"AT": AT_imgs[c], "zh": zh_img, "zl": zl_img}
            for c in range(NCORES)]
    res = _run(ncA, maps)
    x0 = np.concatenate([_unpack(res[c]["po"], 4, C) for c in range(NCORES)], 0)
    x0 = dis0[:, None] * (x0 / az) + 2.0 * dis0[:, None] ** 2 * y0

    # ---- down levels 0,1 (K2, K3)
    Bh = A0 + np.eye(N, dtype=np.float32)
    xcur = x0
    n = N
    xs = [x0]
    Ms, perms, diss = [], [], [dis0]
    for lev in range(2):
        p = np.asarray(p_pool[lev], np.float32)
        score = (xcur @ p) / np.linalg.norm(p)
        k = n // 2
        perm = np.argsort(-score, kind="stable")[:k]
        sv = score[perm]
        perms.append(perm)
        L = Bh[perm, :]
        R = Bh[:, perm]
        assert Bh.max() <= 15
        diagM = np.einsum('ak,ka->a', L, R, optimize=True).astype(np.float32)
        r = R.sum(1, dtype=np.float64)
        deg = (L @ r.astype(np.float32)).astype(np.float64) - diagM + 2.0
        dis = _mk_dis(deg.astype(np.float32))
        diss.append(dis)
        xp = (xcur[perm] * np.tanh(sv)[:, None]).astype(np.float32)
        w = dis[:, None] * (xp @ np.asarray(W_down[lev], np.float32))
        CW, RW = k // 2, k // 4
        nc = ncB0 if lev == 0 else ncB1
        maps = []
        if lev == 0:
            aw = _pow2_for(np.abs(w).max(), 128.0)
        else:
            aw = _pow2_for(np.abs(w).max(), 8192.0)
        for c in range(NCORES):
            i, j = c // 2, c % 2
            m = {"Rc": _pack(np.ascontiguousarray(R[:, j * CW:(j + 1) * CW]), NP8),
                 "LrT": _pack(np.ascontiguousarray(L[i * RW:(i + 1) * RW].T), NP8)}
            wj = w[j * CW:(j + 1) * CW]
            if lev == 0:
                h8, l8 = _split8(wj, aw)
                m["w0"], m["w1"] = _pack(h8, NP8), _pack(l8, NP8)
            else:
                m["w0"] = _pack((wj * aw).astype(np.float32), NP16)
            maps.append(m)
        res = _run(nc, maps)
        # assemble M [k, k] and reduce P partials
        M = np.empty((k, k), np.float32)
        xnew = np.empty((k, C), np.float32)
        for i in range(4):
            Pi = (_unpack(res[2 * i]["Po"].astype(np.float32), RW // 128, C)
                  + _unpack(res[2 * i + 1]["Po"].astype(np.float32),
                            RW // 128, C)) / aw
            sl = slice(i * RW, (i + 1) * RW)
            xnew[sl] = np.maximum(
                dis[sl, None] * (Pi + (2.0 - diagM[sl, None]) * w[sl]), 0.0)
            for j in range(2):
                Xp = _unpack(res[2 * i + j]["XT"].astype(np.float32),
                             CW // 128, RW)
                M[sl, j * CW:(j + 1) * CW] = Xp.T
        if lev == 0:
            assert M.max() <= 15
        else:
            assert M.max() <= 255
        Ms.append(M)
        Bh = M - np.diag(np.diag(M)) + np.eye(k, dtype=np.float32)
        xs.append(xnew)
        xcur = xnew
        n = k

    # ---- level 2 (K4a): factored, no M3
    lev = 2
    p = np.asarray(p_pool[lev], np.float32)
    score = (xcur @ p) / np.linalg.norm(p)
    k = n // 2
    perm = np.argsort(-score, kind="stable")[:k]
    sv = score[perm]
    perms.append(perm)
    L3 = Bh[perm, :]
    R3 = Bh[:, perm]
    assert Bh.max() <= 255
    diagM3 = np.einsum('ak,ka->a', L3, R3, optimize=True).astype(np.float32)
    r = R3.sum(1, dtype=np.float64)
    deg = (L3 @ r.astype(np.float32)).astype(np.float64) - diagM3 + 2.0
    dis3 = _mk_dis(deg.astype(np.float32))
    diss.append(dis3)
    xp = (xcur[perm] * np.tanh(sv)[:, None]).astype(np.float32)
    w3 = dis3[:, None] * (xp @ np.asarray(W_down[2], np.float32))
    aw3 = _pow2_for(np.abs(w3).max(), 8192.0)
    R3_img = _pack(R3, NPBF)
    w3_img = _pack((w3 * aw3).astype(np.float32), NP16)
    ident128 = np.eye(128, dtype=np.float32).astype(NPBF)
    maps = [{"R3": R3_img, "w3": w3_img, "ident": ident128,
             "L3cT": _pack(np.ascontiguousarray(L3[c * 64:(c + 1) * 64].T),
                           NPBF)}
            for c in range(NCORES)]
    res = _run(ncC, maps)
    P3 = np.concatenate([res[c]["xo"] for c in range(NCORES)], 0) / aw3
    x_d2 = np.maximum(dis3[:, None] * (P3 + (2.0 - diagM3[:, None]) * w3), 0.0)

    # ---- K4b
    x_d0, x_d1 = xs[1], xs[2]
    dis1, dis2 = diss[1], diss[2]
    M1, M2 = Ms
    M1p = M1 - np.diag(np.diag(M1)) + 2.0 * np.eye(2048, dtype=np.float32)
    M2p = M2 - np.diag(np.diag(M2)) + 2.0 * np.eye(1024, dtype=np.float32)
    assert M1p.max() <= 15
    up = np.zeros_like(x_d1)
    up[perms[2]] = x_d2
    xa1 = x_d1 + up
    w_u = dis2[:, None] * (xa1 @ np.asarray(W_up[0], np.float32))
    w1 = dis1[:, None] * x_d0
    au = _pow2_for(np.abs(w_u).max(), 8192.0)
    sbase = (dis1[perms[1]] * dis2).astype(np.float32)
    boundX = float(sbase.max() * np.abs(M2p).sum(1).max() * np.abs(w_u).max())
    ac = min(_pow2_for(np.abs(w1).max(), 128.0), _pow2_for(boundX, 8192.0))
    sp = sbase * (ac / au)
    M2pT_img = _pack(np.ascontiguousarray(M2p.T), NPBF)
    wu_img = _pack((w_u * au).astype(np.float32), NP16)
    w1h8, w1l8 = _split8(w1, ac)
    w1h_img, w1l_img = _pack(w1h8, NP8), _pack(w1l8, NP8)
    Wu1_img = _pack(np.asarray(W_up[1], np.float32), NP16)
    sv_img = np.ascontiguousarray(sp.reshape(8, 128).T.astype(np.float32))
    ident = np.eye(128, dtype=np.float32).astype(NPBF)
    maps = []
    for c in range(NCORES):
        sl = slice(c * 256, (c + 1) * 256)
        Kst = np.vstack([M1p[sl].T, M1p[sl][:, perms[1]].T])
        d1 = (dis1[sl] / ac).astype(np.float32)
        maps.append({
            "M2pT": M2pT_img, "wu": wu_img, "w1h": w1h_img, "w1l": w1l_img,
            "Wu1": Wu1_img, "Kst": _pack(np.ascontiguousarray(Kst), NP8),
            "ident": ident, "svec": sv_img,
            "d1vec": np.ascontiguousarray(d1.reshape(2, 128).T)})
    res = _run(ncD, maps)
    xU2 = np.concatenate([_unpack(res[c]["xo"], 2, C) for c in range(NCORES)], 0)

    # ---- K4c (module A again)
    up0 = np.zeros_like(x0)
    up0[perms[0]] = xU2
    g = (x0 + up0) @ np.asarray(W_final, np.float32)
    z2 = dis0[:, None] * g
    a2 = _pow2_for(np.abs(z2).max(), 128.0)
    z2h, z2l = _split8(z2, a2)
    z2h_img, z2l_img = _pack(z2h, NP8), _pack(z2l, NP8)
    maps = [{"AT": AT_imgs[c], "zh": z2h_img, "zl": z2l_img}
            for c in range(NCORES)]
    res = _run(ncA, maps)
    out = np.concatenate([_unpack(res[c]["po"], 4, C) for c in range(NCORES)], 0)
    out = dis0[:, None] * (out / a2) + 2.0 * dis0[:, None] ** 2 * g
    return out.astype(np.float32)


# revision 30
# speedup vs baseline: 1.0239x; 1.0132x over previous
"""GraphUNet (N=4096, E=65536, C=256, depth 3, ratio 0.5) on 8 trn2 NeuronCores.

Five compiled modules, six launches. Device does all adjacency matmuls
(A@x SpMMs and the dense pooled A@A products); host does O(n^2) prep,
top-k, permutation gathers, CxC weight folds, and scaling-vector algebra
(all folded out of the device programs).

  A  (K1+K4c) row-sharded N0-apply: psum = A0[rows] @ z, z host-split into
     two scaled fp8 halves (DoubleRow), raw f32 psums shipped; host applies
     dis scalings + 2*dis^2 diag term.
  B0 (K2) level-1: (4 row x 2 col)-grid M1 = L@R in fp8 DR; M^T col-blocks
     shipped fp8 (ints, exact); partial GCN P = X'^T @ w (fp8 DR) shipped
     f32; host reduces the 2 k-partials, applies dis/diag corrections+relu.
  B1 (K3) level-2: same at half size; M2 entries >16 so X' ships bf16 and
     the P-chain rhs is fp16.
  C  (K4a) level-3 factored GCN (no M3 materialization): u = R3 @ w3,
     x_rows = L3[rows] @ u; bf16/fp16.
  D  (K4b) both up-GCNs fused: xU1 = relu-scale(M2p^T-chain) written
     straight into the combined rhs tile; v2 = Kst^T @ [w1; xU1s] with the
     unpool-scatter folded into host-gathered Kst = [M1p; M1p[:,p2]]^T fp8;
     transpose + Wu1 matmul + relu on device.

All device inputs are host-packed [128, X] contiguous SBUF images (full
DMA bandwidth, no sub-512B descriptor penalty). All fp8/fp16 payloads are
pre-scaled by power-of-2 to dodge fp8's 2^-10 subnormal floor; scales are
folded into host-side post-processing (everything downstream is linear,
and relu commutes with positive scales).
"""

import numpy as np
import ml_dtypes

from contextlib import ExitStack

import concourse.bass as bass
import concourse.mybir as mybir
import concourse.tile as tile
from concourse import bacc
from concourse.bass_utils import run_bass_kernel_spmd

NCORES = 8
C = 256
F32 = mybir.dt.float32
F16 = mybir.dt.float16
BF16 = mybir.dt.bfloat16
FP8 = mybir.dt.float8e4

NP8 = ml_dtypes.float8_e4m3fn
NP16 = np.float16
NPBF = ml_dtypes.bfloat16

_TRACE = {"on": False, "results": [], "ncs": []}


# ------------------------------------------------------------- host helpers
def _pack(arr, np_dt):
    """[K, F] -> [128, (K//128)*F] image; k-tile o holds rows o*128..o*128+127."""
    K, F = arr.shape
    KT = K // 128
    return np.ascontiguousarray(
        arr.reshape(KT, 128, F).transpose(1, 0, 2).reshape(128, KT * F)
    ).astype(np_dt)


def _unpack(img, MO, F):
    """[128, MO*F] -> [MO*128, F] (inverse of _pack on the output side)."""
    return np.ascontiguousarray(
        img.reshape(128, MO, F).transpose(1, 0, 2).reshape(MO * 128, F))


def _pow2_for(m, target):
    m = float(m)
    return 1.0 if m <= 0 else float(2.0 ** np.floor(np.log2(target / m)))


def _split8(x, alpha):
    v = (x * alpha).astype(np.float32)
    h = v.astype(NP8)
    lo = (v - h.astype(np.float32)).astype(NP8)
    return h, lo


def _mk_dis(deg):
    return (1.0 / np.sqrt(np.maximum(deg, 1e-12))).astype(np.float32)


# ----------------------------------------------------------- device helpers
def _in_img(nc, name, KT, F, dt):
    return nc.dram_tensor(name, [128, KT * F], dt, kind="ExternalInput")


def _ld_chunks(nc, pool, dram, KT, F, tag, chunks):
    """Allocate [128, KT, F] tile; return (tile, chunk-issue fns).
    chunks: list of kt-counts per dma, or an int chunk size."""
    t = pool.tile([128, KT, F], dram.dtype, tag=tag, name=tag)
    r = dram.ap().rearrange("p (o f) -> p o f", f=F)
    if isinstance(chunks, int):
        chunks = [chunks] * ((KT + chunks - 1) // chunks)
    fns, k = [], 0
    for ck in chunks:
        k0, k1 = k, min(KT, k + ck)
        k = k1
        fns.append(lambda k0=k0, k1=k1: nc.sync.dma_start(
            t[:, k0:k1, :], r[:, k0:k1, :]))
        if k >= KT:
            break
    return t, fns


def _interleave(*fn_lists):
    n = max(len(f) for f in fn_lists)
    for i in range(n):
        for fns in fn_lists:
            if i < len(fns):
                fns[i]()


def _spread_copy(nc, idx, dst, src):
    if idx % 2 == 0:
        nc.scalar.copy(dst, src)
    else:
        nc.vector.tensor_copy(dst, src)


def _mm_ktouter(nc, ps, lhsT, rhs_list, M, NF, dr, tagp, consumer,
                stagger=False):
    """kt-outer accumulation: psums for all M//128 row-blocks live at once.
    lhsT [128, KT, M]; each rhs [128, KT, NF]. dr: fp8 DoubleRow.
    stagger: issue the last k-step mo-major with the consumer interleaved, so
    psum copies start as soon as each row-block's accumulation closes."""
    n_mo = M // 128
    KT = lhsT.shape[1]
    pss = [ps.tile([128, NF], F32, tag=f"{tagp}{m}", name=f"{tagp}{m}")
           for m in range(n_mo)]

    def mm(mo, k, ci, start, stop):
        if dr:
            nc.tensor.matmul(
                pss[mo][:],
                lhsT[:, 2 * k:2 * k + 2, mo * 128:(mo + 1) * 128],
                rhs_list[ci][:, 2 * k:2 * k + 2, :],
                start=start, stop=stop,
                perf_mode=mybir.MatmulPerfMode.DoubleRow)
        else:
            nc.tensor.matmul(
                pss[mo][:], lhsT[:, k, mo * 128:(mo + 1) * 128],
                rhs_list[ci][:, k, :], start=start, stop=stop)

    nch = len(rhs_list)
    KS = (KT // 2) if dr else KT
    nk_main = KS - 1 if (stagger and KS > 1) else KS
    step = 0
    for k in range(nk_main):
        for ci in range(nch):
            step += 1
            for mo in range(n_mo):
                mm(mo, k, ci, step == 1, step == KS * nch)
    if nk_main < KS:
        for mo in range(n_mo):
            for ci in range(nch):
                mm(mo, KS - 1, ci, False, ci == nch - 1)
            consumer(mo, pss[mo])
    else:
        for mo in range(n_mo):
            consumer(mo, pss[mo])


def _new_nc():
    return bacc.Bacc("TRN2", target_bir_lowering=False, debug=False,
                     num_devices=NCORES)


def _run(nc, in_maps):
    res = run_bass_kernel_spmd(nc, in_maps, list(range(NCORES)),
                               trace=_TRACE["on"])
    _TRACE["ncs"].append(nc)
    if _TRACE["on"]:
        _TRACE["results"].append(res)
    return res.results


# --------------------------------------------------------------- module A
def build_A():
    """psum[rows_c] = A0[rows_c] @ (zh + zl); rows_c = 512-row slab."""
    nc = _new_nc()
    KT, RW = 32, 512
    AT = _in_img(nc, "AT", KT, RW, FP8)
    zh = _in_img(nc, "zh", KT, C, FP8)
    zl = _in_img(nc, "zl", KT, C, FP8)
    po = nc.dram_tensor("po", [128, (RW // 128) * C], F32, kind="ExternalOutput")
    with tile.TileContext(nc) as tc:
        ctx = ExitStack()
        sb = ctx.enter_context(tc.tile_pool(name="sb", bufs=1))
        ps = ctx.enter_context(tc.tile_pool(name="ps", bufs=1, space="PSUM"))
        AT_sb, a_fns = _ld_chunks(nc, sb, AT, KT, RW, "AT", [6, 6, 6, 6, 4, 2, 2])
        zh_sb, h_fns = _ld_chunks(nc, sb, zh, KT, C, "zh", 8)
        zl_sb, l_fns = _ld_chunks(nc, sb, zl, KT, C, "zl", 8)
        _interleave(a_fns, h_fns, l_fns)
        o_sb = sb.tile([128, RW // 128, C], F32, tag="o", name="o")
        ro = po.ap().rearrange("p (o f) -> p o f", f=C)

        def fin(mo, p):
            _spread_copy(nc, mo, o_sb[:, mo, :], p[:])
            if mo % 2 == 1:
                nc.sync.dma_start(ro[:, mo - 1:mo + 1, :],
                                  o_sb[:, mo - 1:mo + 1, :])

        _mm_ktouter(nc, ps, AT_sb, [zh_sb, zl_sb], RW, C, True, "mp", fin,
                    stagger=True)
        ctx.close()
    nc.compile()
    return nc


# --------------------------------------------------------------- module B
def build_B(NPREV, NK, xdt, wsplit):
    """(4 rows x 2 cols) grid core: X' = M^T[cols_j, rows_i] (fp8 DR chain),
    P = X'^T @ w[cols_j] partial GCN. wsplit=2 -> two fp8 rhs (DR);
    wsplit=1 -> one fp16 rhs."""
    nc = _new_nc()
    KT = NPREV // 128
    CW, RW = NK // 2, NK // 4
    MOX, MOP = CW // 128, RW // 128
    Rc = _in_img(nc, "Rc", KT, CW, FP8)
    LrT = _in_img(nc, "LrT", KT, RW, FP8)
    wdt = FP8 if wsplit == 2 else F16
    ws = [_in_img(nc, f"w{i}", MOX, C, wdt) for i in range(wsplit)]
    XT = nc.dram_tensor("XT", [128, MOX * RW], xdt, kind="ExternalOutput")
    Po = nc.dram_tensor("Po", [128, MOP * C], BF16, kind="ExternalOutput")
    rck = [6, 6, 6, 6, 4, 2, 2] if KT == 32 else [4, 4, 4, 2, 2]
    with tile.TileContext(nc) as tc:
        ctx = ExitStack()
        sb = ctx.enter_context(tc.tile_pool(name="sb", bufs=1))
        ps = ctx.enter_context(tc.tile_pool(name="ps", bufs=1, space="PSUM"))
        Rc_sb, r_fns = _ld_chunks(nc, sb, Rc, KT, CW, "Rc", rck)
        LrT_sb, l_fns = _ld_chunks(nc, sb, LrT, KT, RW, "LrT", rck)
        w_sbs, w_fns = [], []
        for i, w in enumerate(ws):
            t, fns = _ld_chunks(nc, sb, w, MOX, C, f"w{i}", MOX)
            w_sbs.append(t)
            w_fns.append(fns)
        _interleave(r_fns, l_fns, *w_fns)
        X_sb = sb.tile([128, MOX, RW], xdt, tag="X", name="X")
        rx = XT.ap().rearrange("p (o f) -> p o f", f=RW)

        def xfin(mo, p):
            _spread_copy(nc, mo, X_sb[:, mo, :], p[:])
            if mo == MOX // 2 - 1 or mo == MOX - 1:
                nc.sync.dma_start(rx[:, mo - MOX // 2 + 1:mo + 1, :],
                                  X_sb[:, mo - MOX // 2 + 1:mo + 1, :])

        _mm_ktouter(nc, ps, Rc_sb, [LrT_sb], CW, RW, True, "mp", xfin,
                    stagger=True)
        P_sb = sb.tile([128, MOP, C], BF16, tag="P", name="P")

        def pfin(mo, p):
            _spread_copy(nc, mo + 1, P_sb[:, mo, :], p[:])

        _mm_ktouter(nc, ps, X_sb, w_sbs, RW, C, wsplit == 2, "mp", pfin,
                    stagger=True)
        nc.sync.dma_start(Po.ap(), P_sb[:].rearrange("p o f -> p (o f)"))
        ctx.close()
    nc.compile()
    return nc


# --------------------------------------------------------------- module C
def build_C():
    """M3c = L3[rows_c] @ R3 (rides the R3 stream), then transpose and
    x[rows_c] = M3c @ w3; 64 rows/core."""
    nc = _new_nc()
    R3 = _in_img(nc, "R3", 8, 512, BF16)
    w3 = _in_img(nc, "w3", 4, C, F16)
    L3cT = _in_img(nc, "L3cT", 8, 64, BF16)
    ident = nc.dram_tensor("ident", [128, 128], BF16, kind="ExternalInput")
    xo = nc.dram_tensor("xo", [64, C], F32, kind="ExternalOutput")
    with tile.TileContext(nc) as tc:
        ctx = ExitStack()
        sb = ctx.enter_context(tc.tile_pool(name="sb", bufs=1))
        ps = ctx.enter_context(tc.tile_pool(name="ps", bufs=1, space="PSUM"))
        L3_sb, l_fns = _ld_chunks(nc, sb, L3cT, 8, 64, "L3cT", 8)
        id_sb = sb.tile([128, 128], BF16, tag="id", name="id")
        R3_sb, r_fns = _ld_chunks(nc, sb, R3, 8, 512, "R3", [2, 2, 2, 1, 1])
        w3_sb, w_fns = _ld_chunks(nc, sb, w3, 4, C, "w3", 4)
        l_fns[0]()
        r_fns[0]()
        nc.sync.dma_start(id_sb[:], ident.ap())
        _interleave(r_fns[1:], w_fns)
        # M3c = L3c @ R3  [64, 512], kt-outer over the R3 stream
        pm = ps.tile([128, 512], F32, tag="pm", name="pm")
        for kt in range(8):
            nc.tensor.matmul(pm[:64, :], L3_sb[:, kt, :], R3_sb[:, kt, :],
                             start=(kt == 0), stop=(kt == 7))
        m3 = sb.tile([128, 512], BF16, tag="m3", name="m3")
        nc.scalar.copy(m3[:64, :], pm[:64, :])
        m3T = sb.tile([128, 4, 64], BF16, tag="m3T", name="m3T")
        for cc in range(4):
            pt = ps.tile([128, 64], BF16, tag=f"pt{cc % 2}", name="pt")
            nc.tensor.transpose(pt[:, :], m3[:64, cc * 128:(cc + 1) * 128],
                                id_sb[:64, :64])
            _spread_copy(nc, cc, m3T[:, cc, :], pt[:, :])
        px = ps.tile([128, C], F32, tag="px", name="px")
        for kt in range(4):
            nc.tensor.matmul(px[:64, :], m3T[:, kt, :], w3_sb[:, kt, :],
                             start=(kt == 0), stop=(kt == 3))
        o_sb = sb.tile([128, C], F32, tag="o", name="o")
        nc.scalar.copy(o_sb[:64, :], px[:64, :])
        nc.sync.dma_start(xo.ap(), o_sb[:64, :])
        ctx.close()
    nc.compile()
    return nc


# --------------------------------------------------------------- module D
def build_D():
    """xU1 = relu-scale(M2p-chain) rides the first stream; v2 xU1s-part is
    computed next and parked in SBUF; the w1-part (fp8 DR) rides the second
    stream; merged, transposed, and pushed through Wu1 + relu."""
    nc = _new_nc()
    M2pT = _in_img(nc, "M2pT", 8, 1024, BF16)
    wu = _in_img(nc, "wu", 8, C, F16)
    Kst = _in_img(nc, "Kst", 24, C, FP8)
    w1h = _in_img(nc, "w1h", 16, C, FP8)
    w1l = _in_img(nc, "w1l", 16, C, FP8)
    Wu1 = _in_img(nc, "Wu1", 2, C, F16)
    ident = nc.dram_tensor("ident", [128, 128], BF16, kind="ExternalInput")
    svec = nc.dram_tensor("svec", [128, 8], F32, kind="ExternalInput")
    d1vec = nc.dram_tensor("d1vec", [128, 2], F32, kind="ExternalInput")
    xo = nc.dram_tensor("xo", [128, 2 * C], F32, kind="ExternalOutput")
    with tile.TileContext(nc) as tc:
        ctx = ExitStack()
        sb = ctx.enter_context(tc.tile_pool(name="sb", bufs=1))
        ps = ctx.enter_context(tc.tile_pool(name="ps", bufs=1, space="PSUM"))
        M2pT_sb, m_fns = _ld_chunks(nc, sb, M2pT, 8, 1024, "M2pT",
                                    [1, 2, 2, 1, 1, 1])
        wu_sb, wu_fns = _ld_chunks(nc, sb, wu, 8, C, "wu", [2, 3, 3])
        Kst_sb = sb.tile([128, 24, C], FP8, tag="Kst", name="Kst")
        rk = Kst.ap().rearrange("p (o f) -> p o f", f=C)
        kst_rng = lambda k0, k1: nc.sync.dma_start(Kst_sb[:, k0:k1, :],
                                                   rk[:, k0:k1, :])
        w1h_sb, wh_fns = _ld_chunks(nc, sb, w1h, 16, C, "w1h", 8)
        w1l_sb, wl_fns = _ld_chunks(nc, sb, w1l, 16, C, "w1l", 8)
        Wu1_sb, wf_fns = _ld_chunks(nc, sb, Wu1, 2, C, "Wu1", 2)
        id_sb = sb.tile([128, 128], BF16, tag="id", name="id")
        sv_sb = sb.tile([128, 8], F32, tag="sv", name="sv")
        d1_sb = sb.tile([128, 2], F32, tag="d1", name="d1")
        rhs_sb = sb.tile([128, 8, C], F16, tag="rhs", name="rhs")

        nc.sync.dma_start(sv_sb[:], svec.ap())
        nc.sync.dma_start(d1_sb[:], d1vec.ap())
        _interleave(m_fns, wu_fns)
        kst_rng(16, 24)
        nc.sync.dma_start(id_sb[:], ident.ap())
        _interleave(wh_fns, wl_fns,
                    [lambda: kst_rng(0, 8), lambda: kst_rng(8, 16)], wf_fns)

        # xU1s -> rhs tile k-tiles 0..7; relu+scale spread over Act and DVE
        def xufin(mo, p):
            if mo % 2 == 0:
                nc.scalar.activation(rhs_sb[:, mo, :], p[:],
                                     mybir.ActivationFunctionType.Relu,
                                     scale=sv_sb[:, mo:mo + 1])
            else:
                nc.vector.tensor_scalar(rhs_sb[:, mo, :], p[:],
                                        sv_sb[:, mo:mo + 1], 0.0,
                                        mybir.AluOpType.mult,
                                        mybir.AluOpType.max)

        _mm_ktouter(nc, ps, M2pT_sb, [wu_sb], 1024, C, False, "mp", xufin,
                    stagger=True)

        # v2 xU1s-part (Kst k-tiles 16..23), parked in SBUF pre-scaled by d1
        v2a = sb.tile([128, 2, C], F32, tag="v2a", name="v2a")
        vps = [ps.tile([128, C], F32, tag=f"mp{m}", name=f"v2p{m}")
               for m in range(2)]
        for kt in range(8):
            for mo in range(2):
                nc.tensor.matmul(
                    vps[mo][:], Kst_sb[:, 16 + kt, mo * 128:(mo + 1) * 128],
                    rhs_sb[:, kt, :], start=(kt == 0), stop=(kt == 7))
        nc.scalar.activation(v2a[:, 0, :], vps[0][:],
                             mybir.ActivationFunctionType.Copy,
                             scale=d1_sb[:, 0:1])
        nc.vector.tensor_scalar_mul(v2a[:, 1, :], vps[1][:], d1_sb[:, 1:2])

        # v2 w1-part (fp8 DR over split w1, Kst k-tiles 0..15), then merge
        v2b = sb.tile([128, 2, C], BF16, tag="v2b", name="v2b")
        v2ps = [ps.tile([128, C], F32, tag=f"mp{m}", name=f"v2q{m}")
                for m in range(2)]
        cnt = 0
        for kp in range(8):
            for rhs in (w1h_sb, w1l_sb):
                cnt += 1
                for mo in range(2):
                    nc.tensor.matmul(
                        v2ps[mo][:],
                        Kst_sb[:, 2 * kp:2 * kp + 2, mo * 128:(mo + 1) * 128],
                        rhs[:, 2 * kp:2 * kp + 2, :],
                        start=(cnt == 1), stop=(cnt == 16),
                        perf_mode=mybir.MatmulPerfMode.DoubleRow)
        for mo in range(2):
            nc.vector.scalar_tensor_tensor(
                v2b[:, mo, :], v2ps[mo][:], d1_sb[:, mo:mo + 1],
                v2a[:, mo, :], mybir.AluOpType.mult, mybir.AluOpType.add)
        v2T = sb.tile([128, 2, C], BF16, tag="v2T", name="v2T")
        for mo in range(2):
            for cc in range(2):
                pst = ps.tile([128, 128], BF16, tag=f"mp{4 + (mo * 2 + cc) % 2}",
                              name="pt")
                nc.tensor.transpose(pst[:], v2b[:, mo, cc * 128:(cc + 1) * 128],
                                    id_sb[:])
                _spread_copy(nc, mo * 2 + cc,
                             v2T[:, cc, mo * 128:(mo + 1) * 128], pst[:])
        o_sb = sb.tile([128, 2, C], F32, tag="o", name="o")
        ro = xo.ap().rearrange("p (o f) -> p o f", f=C)

        def ofin(mo, p):
            if mo % 2 == 0:
                nc.scalar.activation(o_sb[:, mo, :], p[:],
                                     mybir.ActivationFunctionType.Relu)
            else:
                nc.vector.tensor_scalar_max(o_sb[:, mo, :], p[:], 0.0)
            nc.sync.dma_start(ro[:, mo, :], o_sb[:, mo, :])

        _mm_ktouter(nc, ps, v2T, [Wu1_sb], 256, C, False, "mp", ofin,
                    stagger=True)
        ctx.close()
    nc.compile()
    return nc


# =================================================================== host
def kernel(x, edge_index, W_init, b_init, W_down, b_down, p_pool,
           W_up, b_up, W_final, b_final):
    x = np.asarray(x, np.float32)
    N = x.shape[0]

    A0 = np.zeros((N, N), np.float32)
    np.add.at(A0, (np.asarray(edge_index[0]), np.asarray(edge_index[1])), 1.0)
    assert A0.max() <= 15
    dis0 = _mk_dis(A0.sum(1) + 2.0)
    y0 = x @ np.asarray(W_init, np.float32)
    z = dis0[:, None] * y0

    ncA = build_A()
    ncB0 = build_B(4096, 2048, FP8, 2)
    ncB1 = build_B(2048, 1024, BF16, 1)
    ncC = build_C()
    ncD = build_D()

    # per-core A0 row-slab lhsT images (shared by K1 and K4c)
    AT_imgs = [_pack(np.ascontiguousarray(A0[c * 512:(c + 1) * 512].T), NP8)
               for c in range(NCORES)]

    # ---- K1
    az = _pow2_for(np.abs(z).max(), 128.0)
    zh, zl = _split8(z, az)
    zh_img, zl_img = _pack(zh, NP8), _pack(zl, NP8)
    maps = [{"AT": AT_imgs[c], "zh": zh_img, "zl": zl_img}
            for c in range(NCORES)]
    res = _run(ncA, maps)
    x0 = np.concatenate([_unpack(res[c]["po"], 4, C) for c in range(NCORES)], 0)
    x0 = dis0[:, None] * (x0 / az) + 2.0 * dis0[:, None] ** 2 * y0

    # ---- down levels 0,1 (K2, K3)
    Bh = A0 + np.eye(N, dtype=np.float32)
    xcur = x0
    n = N
    xs = [x0]
    Ms, perms, diss = [], [], [dis0]
    for lev in range(2):
        p = np.asarray(p_pool[lev], np.float32)
        score = (xcur @ p) / np.linalg.norm(p)
        k = n // 2
        perm = np.argsort(-score, kind="stable")[:k]
        sv = score[perm]
        perms.append(perm)
        L = Bh[perm, :]
        R = Bh[:, perm]
        assert Bh.max() <= 15
        diagM = np.einsum('ak,ka->a', L, R, optimize=True).astype(np.float32)
        r = R.sum(1, dtype=np.float64)
        deg = (L @ r.astype(np.float32)).astype(np.float64) - diagM + 2.0
        dis = _mk_dis(deg.astype(np.float32))
        diss.append(dis)
        xp = (xcur[perm] * np.tanh(sv)[:, None]).astype(np.float32)
        w = dis[:, None] * (xp @ np.asarray(W_down[lev], np.float32))
        CW, RW = k // 2, k // 4
        nc = ncB0 if lev == 0 else ncB1
        maps = []
        if lev == 0:
            aw = _pow2_for(np.abs(w).max(), 128.0)
        else:
            aw = _pow2_for(np.abs(w).max(), 8192.0)
        for c in range(NCORES):
            i, j = c // 2, c % 2
            m = {"Rc": _pack(np.ascontiguousarray(R[:, j * CW:(j + 1) * CW]), NP8),
                 "LrT": _pack(np.ascontiguousarray(L[i * RW:(i + 1) * RW].T), NP8)}
            wj = w[j * CW:(j + 1) * CW]
            if lev == 0:
                h8, l8 = _split8(wj, aw)
                m["w0"], m["w1"] = _pack(h8, NP8), _pack(l8, NP8)
            else:
                m["w0"] = _pack((wj * aw).astype(np.float32), NP16)
            maps.append(m)
        res = _run(nc, maps)
        # assemble M [k, k] and reduce P partials
        M = np.empty((k, k), np.float32)
        xnew = np.empty((k, C), np.float32)
        for i in range(4):
            Pi = (_unpack(res[2 * i]["Po"].astype(np.float32), RW // 128, C)
                  + _unpack(res[2 * i + 1]["Po"].astype(np.float32),
                            RW // 128, C)) / aw
            sl = slice(i * RW, (i + 1) * RW)
            xnew[sl] = np.maximum(
                dis[sl, None] * (Pi + (2.0 - diagM[sl, None]) * w[sl]), 0.0)
            for j in range(2):
                Xp = _unpack(res[2 * i + j]["XT"].astype(np.float32),
                             CW // 128, RW)
                M[sl, j * CW:(j + 1) * CW] = Xp.T
        if lev == 0:
            assert M.max() <= 15
        else:
            assert M.max() <= 255
        Ms.append(M)
        Bh = M - np.diag(np.diag(M)) + np.eye(k, dtype=np.float32)
        xs.append(xnew)
        xcur = xnew
        n = k

    # ---- level 2 (K4a): factored, no M3
    lev = 2
    p = np.asarray(p_pool[lev], np.float32)
    score = (xcur @ p) / np.linalg.norm(p)
    k = n // 2
    perm = np.argsort(-score, kind="stable")[:k]
    sv = score[perm]
    perms.append(perm)
    L3 = Bh[perm, :]
    R3 = Bh[:, perm]
    assert Bh.max() <= 255
    diagM3 = np.einsum('ak,ka->a', L3, R3, optimize=True).astype(np.float32)
    r = R3.sum(1, dtype=np.float64)
    deg = (L3 @ r.astype(np.float32)).astype(np.float64) - diagM3 + 2.0
    dis3 = _mk_dis(deg.astype(np.float32))
    diss.append(dis3)
    xp = (xcur[perm] * np.tanh(sv)[:, None]).astype(np.float32)
    w3 = dis3[:, None] * (xp @ np.asarray(W_down[2], np.float32))
    aw3 = _pow2_for(np.abs(w3).max(), 8192.0)
    R3_img = _pack(R3, NPBF)
    w3_img = _pack((w3 * aw3).astype(np.float32), NP16)
    ident128 = np.eye(128, dtype=np.float32).astype(NPBF)
    maps = [{"R3": R3_img, "w3": w3_img, "ident": ident128,
             "L3cT": _pack(np.ascontiguousarray(L3[c * 64:(c + 1) * 64].T),
                           NPBF)}
            for c in range(NCORES)]
    res = _run(ncC, maps)
    P3 = np.concatenate([res[c]["xo"] for c in range(NCORES)], 0) / aw3
    x_d2 = np.maximum(dis3[:, None] * (P3 + (2.0 - diagM3[:, None]) * w3), 0.0)

    # ---- K4b
    x_d0, x_d1 = xs[1], xs[2]
    dis1, dis2 = diss[1], diss[2]
    M1, M2 = Ms
    M1p = M1 - np.diag(np.diag(M1)) + 2.0 * np.eye(2048, dtype=np.float32)
    M2p = M2 - np.diag(np.diag(M2)) + 2.0 * np.eye(1024, dtype=np.float32)
    assert M1p.max() <= 15
    up = np.zeros_like(x_d1)
    up[perms[2]] = x_d2
    xa1 = x_d1 + up
    w_u = dis2[:, None] * (xa1 @ np.asarray(W_up[0], np.float32))
    w1 = dis1[:, None] * x_d0
    au = _pow2_for(np.abs(w_u).max(), 8192.0)
    sbase = (dis1[perms[1]] * dis2).astype(np.float32)
    boundX = float(sbase.max() * np.abs(M2p).sum(1).max() * np.abs(w_u).max())
    ac = min(_pow2_for(np.abs(w1).max(), 128.0), _pow2_for(boundX, 8192.0))
    sp = sbase * (ac / au)
    M2pT_img = _pack(np.ascontiguousarray(M2p.T), NPBF)
    wu_img = _pack((w_u * au).astype(np.float32), NP16)
    w1h8, w1l8 = _split8(w1, ac)
    w1h_img, w1l_img = _pack(w1h8, NP8), _pack(w1l8, NP8)
    Wu1_img = _pack(np.asarray(W_up[1], np.float32), NP16)
    sv_img = np.ascontiguousarray(sp.reshape(8, 128).T.astype(np.float32))
    ident = np.eye(128, dtype=np.float32).astype(NPBF)
    maps = []
    for c in range(NCORES):
        sl = slice(c * 256, (c + 1) * 256)
        Kst = np.vstack([M1p[sl].T, M1p[sl][:, perms[1]].T])
        d1 = (dis1[sl] / ac).astype(np.float32)
        maps.append({
            "M2pT": M2pT_img, "wu": wu_img, "w1h": w1h_img, "w1l": w1l_img,
            "Wu1": Wu1_img, "Kst": _pack(np.ascontiguousarray(Kst), NP8),
            "ident": ident, "svec": sv_img,
            "d1vec": np.ascontiguousarray(d1.reshape(2, 128).T)})
    res = _run(ncD, maps)
    xU2 = np.concatenate([_unpack(res[c]["xo"], 2, C) for c in range(NCORES)], 0)

    # ---- K4c (module A again)
    up0 = np.zeros_like(x0)
    up0[perms[0]] = xU2
    g = (x0 + up0) @ np.asarray(W_final, np.float32)
    z2 = dis0[:, None] * g
    a2 = _pow2_for(np.abs(z2).max(), 128.0)
    z2h, z2l = _split8(z2, a2)
    z2h_img, z2l_img = _pack(z2h, NP8), _pack(z2l, NP8)
    maps = [{"AT": AT_imgs[c], "zh": z2h_img, "zl": z2l_img}
            for c in range(NCORES)]
    res = _run(ncA, maps)
    out = np.concatenate([_unpack(res[c]["po"], 4, C) for c in range(NCORES)], 0)
    out = dis0[:, None] * (out / a2) + 2.0 * dis0[:, None] ** 2 * g
    return out.astype(np.float32)
